# revision 2
# baseline (speedup 1.0000x reference)
"""ContextNet dynamic-conv kernel for 8 TRN2 NeuronCores.

Math: the reference computes, per sample b:
    gap[b]  = x[b].sum(T) / len[b]                  (C,)
    h[b]    = sigmoid(gap[b] @ w1.T + b1)           (2C,)
    w_dyn[b, co, ci, k] = h[b, 2*co + (ci>=C/2)] * W[co, ci, k]
        where W = w2.reshape(C, C, K)               (static across batch!)
    out[b]  = conv1d(x[b], w_dyn[b], pad=K//2)      (C, T)

Key structure: the per-sample weight is a static tensor W scaled by
h-factors that depend only on (output channel, input-channel half):
    out[b] = h0[co] * conv(x[b,:64], W[:, :64]) + h1[co] * conv(x[b,64:], W[:, 64:])

Two conv paths exploit this:
  * normal path: scale the pre-transposed W by S_b once per sample (bf16)
    and run the conv as 5 shifted full-128-contraction bf16 matmuls per
    512-col tile, accumulating in PSUM.
  * static path (pipeline fill only): sample 0's first S_TILES tiles are
    computed with the UNSCALED weight halves as two concurrent 64-deep
    row-tiled matmuls (tile_position (0,0)/(64,0)), staged to SBUF in
    f32, and combined with h0/h1 per-output-channel AFTER h is ready.
    This lets the PE start as soon as the first x chunk lands instead of
    waiting for the full sample + h chain, and warms the HAM clock gate.

x ships from the host as bf16 (the conv is bf16 anyway): halves input
HBM traffic and the sample-0 critical load. Row sums for the GAP run on
the (otherwise idle) DVE. Output leaves the device as bf16; host widens.

Sharding: pure data parallel over batch B=32 -> 4 samples per core x 8.
"""

import numpy as np
from contextlib import ExitStack

import concourse.bacc as bacc
import concourse.tile as tile
from concourse import mybir
from concourse.bass_utils import run_bass_kernel_spmd

B, C, T = 32, 128, 8192
K = 5
PAD = (K - 1) // 2
NCORES = 8
BL = B // NCORES          # samples per core
TT = 512                  # conv tile width (one PSUM bank of f32)
NTILES = T // TT
GRP = 4                   # conv tiles batched per output DMA (512 KiB bf16)
S_TILES = 8               # sample-0 tiles on the static (split-conv) path
# sample-0 input chunk bounds (halo-aligned to 512*j + 4 so static tiles
# unlock as chunks land; later samples load in one 2 MiB DMA)
CH0 = [0, 1028, 3076, 5636, 7684, 8192]
NCH0 = len(CH0) - 1
N_WARM = 5                # dummy matmuls to start warming the PE clock gate

FP32 = mybir.dt.float32
BF16 = mybir.dt.bfloat16

AF = mybir.ActivationFunctionType
ALU = mybir.AluOpType
AXL = mybir.AxisListType


def build_nc():
    nc = bacc.Bacc("TRN2", target_bir_lowering=False, debug=False)

    x_d = nc.dram_tensor("x", [BL, C, T], BF16, kind="ExternalInput").ap()
    il_d = nc.dram_tensor("invlen", [1, BL], FP32, kind="ExternalInput").ap()
    w1t_d = nc.dram_tensor("w1t", [C, 2 * C], BF16, kind="ExternalInput").ap()
    b1_d = nc.dram_tensor("b1", [1, 2 * C], FP32, kind="ExternalInput").ap()
    wt_d = nc.dram_tensor("wt", [C, K * C], FP32, kind="ExternalInput").ap()
    wtbf_d = nc.dram_tensor("wtbf", [C, K * C], BF16, kind="ExternalInput").ap()
    ones_d = nc.dram_tensor("ones", [1, 64], BF16, kind="ExternalInput").ap()
    out_d = nc.dram_tensor("out", [BL, C, T], BF16, kind="ExternalOutput").ap()

    with ExitStack() as ctx:
        tc = ctx.enter_context(tile.TileContext(nc))

        const = ctx.enter_context(tc.tile_pool(name="const", bufs=1))
        xb = ctx.enter_context(tc.tile_pool(name="xb", bufs=2))
        statp = ctx.enter_context(tc.tile_pool(name="statp", bufs=S_TILES))
        outp = ctx.enter_context(tc.tile_pool(name="outp", bufs=3))
        small = ctx.enter_context(tc.tile_pool(name="small", bufs=3))
        wscp = ctx.enter_context(tc.tile_pool(name="wscp", bufs=2))
        pconv = ctx.enter_context(tc.tile_pool(name="pconv", bufs=3, space="PSUM"))
        psp = ctx.enter_context(tc.tile_pool(name="psp", bufs=1, space="PSUM"))
        php = ctx.enter_context(tc.tile_pool(name="php", bufs=1, space="PSUM"))

        # constants ride the ACT HWDGE ring; wtbf first (static MMs need it
        # earliest), wt (only needed for the first weight scaling) last
        dz = const.tile([C, TT], BF16)
        nc.vector.memset(dz[:], 0.0)
        wtbf_sb = const.tile([C, K * C], BF16)
        nc.scalar.dma_start(wtbf_sb[:], wtbf_d[:])
        w1t_sb = const.tile([C, 2 * C], BF16)
        nc.scalar.dma_start(w1t_sb[:], w1t_d[:])
        b1_sb = const.tile([1, 2 * C], FP32)
        nc.scalar.dma_start(b1_sb[:], b1_d[:])
        il_sb = const.tile([1, BL], FP32)
        nc.scalar.dma_start(il_sb[:], il_d[:])
        ones_sb = const.tile([1, 64], BF16)
        nc.scalar.dma_start(ones_sb[:], ones_d[:])
        wt_sb = const.tile([C, K * C], FP32)
        nc.scalar.dma_start(wt_sb[:], wt_d[:])

        # HAM warm-up: matmuls on zeroed SBUF into the psp scratch bank.
        # They retire before the first real MM's data can land, and start
        # the PE activity window ~2 us earlier.
        pwarm = psp.tile([C, TT], FP32, tag="s")
        for _ in range(N_WARM):
            nc.tensor.matmul(pwarm[:], lhsT=dz[:, 0:C], rhs=dz[:], start=True, stop=True)

        def emit_load0():
            """Sample 0: chunked DMA (critical path) + per-chunk row sums."""
            x_b = xb.tile([C, T + 2 * PAD], BF16, tag="xb")
            nc.vector.memset(x_b[:, 0:PAD], 0.0)
            nc.vector.memset(x_b[:, T + PAD : T + 2 * PAD], 0.0)
            gap_parts = small.tile([C, NCH0], FP32, tag="gapp")
            for c in range(NCH0):
                lo, hi = CH0[c], CH0[c + 1]
                nc.sync.dma_start(x_b[:, PAD + lo : PAD + hi], x_d[0, :, lo:hi])
            for c in range(NCH0):
                lo, hi = CH0[c], CH0[c + 1]
                nc.vector.tensor_reduce(
                    gap_parts[:, c : c + 1],
                    x_b[:, PAD + lo : PAD + hi],
                    axis=AXL.X,
                    op=ALU.add,
                )
            gap_r = small.tile([C, 1], FP32, tag="gapr")
            nc.vector.tensor_reduce(gap_r[:], gap_parts[:], axis=AXL.X, op=ALU.add)
            gap_bf = small.tile([C, 1], BF16, tag="gapbf")
            nc.vector.tensor_copy(gap_bf[:], gap_r[:])
            return x_b, gap_bf

        def emit_load(b):
            """Samples 1..: one 2 MiB DMA + a two-step row sum on DVE."""
            x_b = xb.tile([C, T + 2 * PAD], BF16, tag="xb")
            nc.vector.memset(x_b[:, 0:PAD], 0.0)
            nc.vector.memset(x_b[:, T + PAD : T + 2 * PAD], 0.0)
            nc.sync.dma_start(x_b[:, PAD : PAD + T], x_d[b, :, 0:T])
            gap_parts = small.tile([C, 2], FP32, tag="gapp")
            x3 = x_b[:, PAD : PAD + T].rearrange("p (a x) -> p a x", a=2)
            nc.vector.tensor_reduce(gap_parts[:], x3, axis=AXL.X, op=ALU.add)
            gap_r = small.tile([C, 1], FP32, tag="gapr")
            nc.vector.tensor_reduce(gap_r[:], gap_parts[:], axis=AXL.X, op=ALU.add)
            gap_bf = small.tile([C, 1], BF16, tag="gapbf")
            nc.vector.tensor_copy(gap_bf[:], gap_r[:])
            return x_b, gap_bf

        def emit_weights(b, gap_bf, want_hcols=False):
            """h = sigmoid(gap @ w1.T * invlen + b1); S broadcast; scale W."""
            h_ps = php.tile([1, 2 * C], FP32, tag="h")
            nc.tensor.matmul(
                h_ps[:], lhsT=gap_bf[:], rhs=w1t_sb[:], start=True, stop=True
            )
            h_pre = small.tile([1, 2 * C], FP32, tag="hpre")
            nc.vector.scalar_tensor_tensor(
                h_pre[:], h_ps[:], il_sb[0:1, b : b + 1], b1_sb[:],
                op0=ALU.mult, op1=ALU.add,
            )
            h_sb = small.tile([1, 2 * C], BF16, tag="h")
            nc.scalar.activation(h_sb[:], h_pre[:], AF.Sigmoid)

            h3 = h_sb[:].rearrange("p (a two) -> p two a", two=2)  # (1, 2, 128)
            s_ps = psp.tile([C, 130], FP32, tag="s")
            if want_hcols:
                # h as per-output-channel columns for the static combine;
                # emitted before the S matmuls so the PE is done with this
                # bank by the time DVE reads it
                nc.tensor.matmul(
                    s_ps[:, 128:129], lhsT=h3[:, 0, :], rhs=ones_sb[0:1, 0:1],
                    start=True, stop=True,
                )
                nc.tensor.matmul(
                    s_ps[:, 129:130], lhsT=h3[:, 1, :], rhs=ones_sb[0:1, 0:1],
                    start=True, stop=True,
                )
            # S_b[ci, co] = h[2co + (ci>=64)] via contract-1 broadcast
            nc.tensor.matmul(
                s_ps[0:64, 0:128], lhsT=ones_sb[:], rhs=h3[:, 0, :],
                start=True, stop=True,
            )
            nc.tensor.matmul(
                s_ps[64:128, 0:128], lhsT=ones_sb[:], rhs=h3[:, 1, :],
                start=True, stop=True,
            )
            wsc = wscp.tile([C, K * C], BF16, tag="wsc")
            for k in range(K):
                nc.vector.tensor_mul(
                    wsc[:, k * C : (k + 1) * C],
                    wt_sb[:, k * C : (k + 1) * C],
                    s_ps[:, 0:128],
                )
            return wsc, s_ps

        def emit_static_tile(t, x_b):
            """One 512-col tile of sample 0 via two concurrent 64-deep
            row-tiled matmul groups with static weight halves."""
            pc = pconv.tile([C, 2 * TT], FP32, tag="pc")
            base = t * TT
            for k in range(K):
                nc.tensor.matmul(
                    pc[:, 0:TT],
                    lhsT=wtbf_sb[0:64, k * C : (k + 1) * C],
                    rhs=x_b[0:64, base + k : base + k + TT],
                    start=(k == 0), stop=(k == K - 1),
                )
            for k in range(K):
                nc.tensor.matmul(
                    pc[:, TT : 2 * TT],
                    lhsT=wtbf_sb[64:128, k * C : (k + 1) * C],
                    rhs=x_b[64:128, base + k : base + k + TT],
                    start=(k == 0), stop=(k == K - 1),
                )
            st = statp.tile([C, 2 * TT], FP32, tag="st")
            nc.scalar.copy(st[:], pc[:])
            return st

        def emit_combine(st, s_ps, o_sb, off):
            """o_sb[:, off:off+TT] = h0*conv0 + h1*conv1 (per-partition h)."""
            ctmp = small.tile([C, TT], FP32, tag="ctmp")
            nc.vector.tensor_scalar(
                ctmp[:], st[:, TT : 2 * TT], s_ps[:, 129:130], None, op0=ALU.mult
            )
            nc.vector.scalar_tensor_tensor(
                o_sb[:, off : off + TT], st[:, 0:TT], s_ps[:, 128:129], ctmp[:],
                op0=ALU.mult, op1=ALU.add,
            )

        def emit_conv(b, x_b, wsc, start_tile=0):
            """5 shifted matmuls per 512-tile; 2 PSUM banks per ACT copy."""
            last = b == BL - 1
            for g in range(NTILES // GRP):
                t0g = g * GRP
                if t0g + GRP <= start_tile:
                    continue
                o_sb = outp.tile([C, GRP * TT], BF16, tag="osb")
                for jj in range(max(t0g, start_tile), t0g + GRP, 2):
                    pc = pconv.tile([C, 2 * TT], FP32, tag="pc")
                    for half in range(2):
                        j = jj + half
                        for k in range(K):
                            nc.tensor.matmul(
                                pc[:, half * TT : (half + 1) * TT],
                                lhsT=wsc[:, k * C : (k + 1) * C],
                                rhs=x_b[:, j * TT + k : j * TT + k + TT],
                                start=(k == 0),
                                stop=(k == K - 1),
                            )
                    off = (jj - t0g) * TT
                    nc.scalar.copy(o_sb[:, off : off + 2 * TT], pc[:])
                    if last and g == NTILES // GRP - 1:
                        # drain the final sample's output in 2-tile pieces so
                        # the kernel tail isn't gated on one large DMA
                        nc.scalar.dma_start(
                            out_d[b, :, t0g * TT + off : t0g * TT + off + 2 * TT],
                            o_sb[:, off : off + 2 * TT],
                        )
                if not (last and g == NTILES // GRP - 1):
                    nc.scalar.dma_start(
                        out_d[b, :, t0g * TT : (t0g + GRP) * TT], o_sb[:]
                    )

        # ---- sample 0: chunked load, static tiles, h chain, combines ----
        x0, gap0 = emit_load0()
        sts = [emit_static_tile(t, x0) for t in range(S_TILES)]
        wsc0, sps0 = emit_weights(0, gap0, want_hcols=True)
        for g in range(S_TILES // GRP):
            o_sb = outp.tile([C, GRP * TT], BF16, tag="osb")
            for t in range(g * GRP, (g + 1) * GRP):
                emit_combine(sts[t], sps0, o_sb, (t - g * GRP) * TT)
            nc.scalar.dma_start(out_d[0, :, g * GRP * TT : (g + 1) * GRP * TT], o_sb[:])

        # ---- software pipeline, one sample deep (as before) ----
        x_cur, wsc_cur = x0, wsc0
        for b in range(BL):
            nxt = None
            if b + 1 < BL:
                x_n, gap_n = emit_load(b + 1)
                wsc_n, _ = emit_weights(b + 1, gap_n)
                nxt = (x_n, wsc_n)
            emit_conv(b, x_cur, wsc_cur, start_tile=S_TILES if b == 0 else 0)
            if nxt is not None:
                x_cur, wsc_cur = nxt

    nc.compile()
    return nc


_NC_CACHE = None


def _get_nc():
    global _NC_CACHE
    if _NC_CACHE is None:
        _NC_CACHE = build_nc()
    return _NC_CACHE


def make_in_maps(x, input_lengths, w1, b1, w2):
    import ml_dtypes

    xbf = np.asarray(x, dtype=np.float32).astype(ml_dtypes.bfloat16)
    lens = np.asarray(input_lengths).astype(np.float64)
    invlen = (1.0 / lens).astype(np.float32)
    w1t = np.ascontiguousarray(
        np.asarray(w1, dtype=np.float32).T.astype(ml_dtypes.bfloat16)
    )  # (C, 2C) bf16
    b1r = np.asarray(b1, dtype=np.float32).reshape(1, 2 * C)
    # wt[ci, k*C + co] = W[co, ci, k],  W = w2.reshape(C, C, K)
    wt = np.ascontiguousarray(
        np.asarray(w2, dtype=np.float32)
        .reshape(C, C, K)
        .transpose(1, 2, 0)
        .reshape(C, K * C)
    )
    wtbf = wt.astype(ml_dtypes.bfloat16)
    ones = np.ones((1, 64), dtype=ml_dtypes.bfloat16)

    in_maps = []
    for i in range(NCORES):
        sl = slice(i * BL, (i + 1) * BL)
        in_maps.append(
            {
                "x": np.ascontiguousarray(xbf[sl]),
                "invlen": np.ascontiguousarray(invlen[sl].reshape(1, BL)),
                "w1t": w1t,
                "b1": b1r,
                "wt": wt,
                "wtbf": wtbf,
                "ones": ones,
            }
        )
    return in_maps


def kernel(x, input_lengths, w1, b1, w2, _trace=False):
    nc = _get_nc()
    in_maps = make_in_maps(x, input_lengths, w1, b1, w2)
    res = run_bass_kernel_spmd(nc, in_maps, core_ids=list(range(NCORES)), trace=_trace)
    out = np.concatenate(
        [res.results[i]["out"].astype(np.float32) for i in range(NCORES)], axis=0
    )
    if _trace:
        kernel.last_exec_time_ns = res.exec_time_ns
        kernel.last_results = res
    return out


# revision 9
# speedup vs baseline: 1.0402x; 1.0402x over previous
"""ContextNet dynamic-conv kernel for 8 TRN2 NeuronCores.

Math: the reference computes, per sample b:
    gap[b]  = x[b].sum(T) / len[b]                  (C,)
    h[b]    = sigmoid(gap[b] @ w1.T + b1)           (2C,)
    w_dyn[b, co, ci, k] = h[b, 2*co + (ci>=C/2)] * W[co, ci, k]
        where W = w2.reshape(C, C, K)               (static across batch!)
    out[b]  = conv1d(x[b], w_dyn[b], pad=K//2)      (C, T)

Key structure: the per-sample weight is a static tensor W scaled by
h-factors that depend only on (output channel, input-channel half):
    out[b] = h0[co] * conv(x[b,:64], W[:, :64]) + h1[co] * conv(x[b,64:], W[:, 64:])

Two conv paths exploit this:
  * normal path: scale the pre-transposed W by S_b once per sample (bf16)
    and run the conv as 5 shifted full-128-contraction bf16 matmuls per
    512-col tile, accumulating in PSUM.
  * static path (pipeline fill only): sample 0's first S_TILES tiles are
    computed with the UNSCALED weight halves as two interleaved 64-deep
    row-tiled matmul groups (tile_position (0,0)/(64,0), concurrent in
    the PE sub-arrays), staged to SBUF in f32, and combined with h per
    output channel AFTER h is ready:
        out = (st0 + (h1/h0)*st1) * h0
    one DVE scalar_tensor_tensor + one ACT scaled-copy per tile.  The PE
    starts as soon as the first x chunk lands instead of waiting for the
    full sample + h chain, and warms the HAM clock gate.

GAP row sums cost ~1 ns/elem/lane on every engine (no engine is faster),
so each sample's 8192-elem sum is split between DVE (tensor_reduce) and
ACT (activation Copy + accum_out).  All h-chain and helper ops are
hand-placed into the per-engine FIFOs (instruction queues are strict
program order) so no queue ever stalls another through backpressure.

x ships from the host as bf16 (the conv is bf16 anyway): halves input
HBM traffic and the sample-0 critical load.  Output leaves as bf16.

Sharding: pure data parallel over batch B=32 -> 4 samples per core x 8.
"""

import numpy as np
from contextlib import ExitStack

import concourse.bacc as bacc
import concourse.tile as tile
from concourse import mybir
from concourse.bass_utils import run_bass_kernel_spmd

B, C, T = 32, 128, 8192
K = 5
PAD = (K - 1) // 2
NCORES = 8
BL = B // NCORES          # samples per core
TT = 512                  # conv tile width (one PSUM bank of f32)
NTILES = T // TT
GRP = 4                   # conv tiles batched per output DMA (512 KiB bf16)
S_TILES = 8               # sample-0 tiles on the static (split-conv) path
# sample-0 input chunk bounds (halo-aligned to 512*j + 4 so static tiles
# unlock as chunks land; later samples load in one 2 MiB DMA)
CH0 = [0, 1028, 3076, 5636, 7172, 7684, 8192]
CH0_ACT = (1, 3)          # chunks row-summed on ACT (rest on DVE)
SPLIT = 6144              # b>=1 row-sum split: [0:SPLIT) DVE, [SPLIT:T) ACT
N_WARM = 5                # dummy matmuls to start warming the PE clock gate

FP32 = mybir.dt.float32
BF16 = mybir.dt.bfloat16

AF = mybir.ActivationFunctionType
ALU = mybir.AluOpType
AXL = mybir.AxisListType


def build_nc():
    nc = bacc.Bacc("TRN2", target_bir_lowering=False, debug=False)

    x_d = nc.dram_tensor("x", [BL, C, T], BF16, kind="ExternalInput").ap()
    il_d = nc.dram_tensor("invlen", [1, BL], FP32, kind="ExternalInput").ap()
    w1t_d = nc.dram_tensor("w1t", [C, 2 * C], BF16, kind="ExternalInput").ap()
    b1_d = nc.dram_tensor("b1", [1, 2 * C], FP32, kind="ExternalInput").ap()
    wt_d = nc.dram_tensor("wt", [C, K * C], FP32, kind="ExternalInput").ap()
    wtbf_d = nc.dram_tensor("wtbf", [C, K * C], BF16, kind="ExternalInput").ap()
    ones_d = nc.dram_tensor("ones", [1, 64], BF16, kind="ExternalInput").ap()
    out_d = nc.dram_tensor("out", [BL, C, T], BF16, kind="ExternalOutput").ap()

    with ExitStack() as ctx:
        tc = ctx.enter_context(tile.TileContext(nc))

        const = ctx.enter_context(tc.tile_pool(name="const", bufs=1))
        xb = ctx.enter_context(tc.tile_pool(name="xb", bufs=2))
        statp = ctx.enter_context(tc.tile_pool(name="statp", bufs=S_TILES))
        outp = ctx.enter_context(tc.tile_pool(name="outp", bufs=6))
        small = ctx.enter_context(tc.tile_pool(name="small", bufs=3))
        wscp = ctx.enter_context(tc.tile_pool(name="wscp", bufs=2))
        pconv = ctx.enter_context(tc.tile_pool(name="pconv", bufs=3, space="PSUM"))
        psp = ctx.enter_context(tc.tile_pool(name="psp", bufs=1, space="PSUM"))
        php = ctx.enter_context(tc.tile_pool(name="php", bufs=1, space="PSUM"))

        # constants ride the ACT HWDGE ring; wtbf first (static MMs need it
        # earliest), wt (only needed for the first weight scaling) last
        dz = const.tile([C, TT], BF16)
        nc.vector.memset(dz[:], 0.0)
        wtbf_sb = const.tile([C, K * C], BF16)
        nc.scalar.dma_start(wtbf_sb[:], wtbf_d[:])
        w1t_sb = const.tile([C, 2 * C], BF16)
        nc.scalar.dma_start(w1t_sb[:], w1t_d[:])
        b1_sb = const.tile([1, 2 * C], FP32)
        nc.scalar.dma_start(b1_sb[:], b1_d[:])
        il_sb = const.tile([1, BL], FP32)
        nc.scalar.dma_start(il_sb[:], il_d[:])
        ones_sb = const.tile([1, 64], BF16)
        nc.scalar.dma_start(ones_sb[:], ones_d[:])
        wt_sb = const.tile([C, K * C], FP32)
        nc.scalar.dma_start(wt_sb[:], wt_d[:])
        trash = const.tile([C, 2048], BF16)   # ACT accum_out row-sum byproduct

        # HAM warm-up: matmuls on zeroed SBUF into the psp scratch bank.
        pwarm = psp.tile([C, TT], FP32, tag="s")
        for _ in range(N_WARM):
            nc.tensor.matmul(pwarm[:], lhsT=dz[:, 0:C], rhs=dz[:], start=True, stop=True)

        # ---------------- schedulable pieces ----------------
        def sum_dve(parts, col, x_b, lo, hi):
            nc.vector.tensor_reduce(
                parts[:, col : col + 1], x_b[:, PAD + lo : PAD + hi],
                axis=AXL.X, op=ALU.add,
            )

        def sum_act(parts, col, x_b, lo, hi):
            nc.scalar.activation(
                trash[:, 0 : hi - lo], x_b[:, PAD + lo : PAD + hi], AF.Copy,
                accum_out=parts[:, col : col + 1],
            )

        def emit_gap_finalize(parts):
            gap_r = small.tile([C, 1], FP32, tag="gapr")
            nc.vector.tensor_reduce(gap_r[:], parts[:], axis=AXL.X, op=ALU.add)
            gap_bf = small.tile([C, 1], BF16, tag="gapbf")
            nc.vector.tensor_copy(gap_bf[:], gap_r[:])
            return gap_bf

        def emit_h_matmul(gap_bf):
            h_ps = php.tile([1, 2 * C], FP32, tag="h")
            nc.tensor.matmul(
                h_ps[:], lhsT=gap_bf[:], rhs=w1t_sb[:], start=True, stop=True
            )
            return h_ps

        def emit_h_pre(b, h_ps):
            h_pre = small.tile([1, 2 * C], FP32, tag="hpre")
            nc.vector.scalar_tensor_tensor(
                h_pre[:], h_ps[:], il_sb[0:1, b : b + 1], b1_sb[:],
                op0=ALU.mult, op1=ALU.add,
            )
            return h_pre

        def emit_sigmoid(h_pre):
            h_sb = small.tile([1, 2 * C], BF16, tag="h")
            nc.scalar.activation(h_sb[:], h_pre[:], AF.Sigmoid)
            return h_sb

        def emit_S_wsc(h_sb, want_hcols=False):
            """S broadcast matmuls + weight scaling (h columns first so the
            PE is done with the bank before DVE reads it)."""
            h3 = h_sb[:].rearrange("p (a two) -> p two a", two=2)  # (1, 2, 128)
            s_ps = psp.tile([C, 130], FP32, tag="s")
            if want_hcols:
                nc.tensor.matmul(
                    s_ps[:, 128:129], lhsT=h3[:, 0, :], rhs=ones_sb[0:1, 0:1],
                    start=True, stop=True,
                )
                nc.tensor.matmul(
                    s_ps[:, 129:130], lhsT=h3[:, 1, :], rhs=ones_sb[0:1, 0:1],
                    start=True, stop=True,
                )
            # S_b[ci, co] = h[2co + (ci>=64)] via contract-1 broadcast
            nc.tensor.matmul(
                s_ps[0:64, 0:128], lhsT=ones_sb[:], rhs=h3[:, 0, :],
                start=True, stop=True,
            )
            nc.tensor.matmul(
                s_ps[64:128, 0:128], lhsT=ones_sb[:], rhs=h3[:, 1, :],
                start=True, stop=True,
            )
            wsc = wscp.tile([C, K * C], BF16, tag="wsc")
            for k in range(K):
                nc.vector.tensor_mul(
                    wsc[:, k * C : (k + 1) * C],
                    wt_sb[:, k * C : (k + 1) * C],
                    s_ps[:, 0:128],
                )
            return wsc, s_ps

        def emit_static_tile(t, x_b):
            """One 512-col tile of sample 0 via two interleaved 64-deep
            row-tiled matmul groups (concurrent PE sub-arrays)."""
            pc = pconv.tile([C, 2 * TT], FP32, tag="pc")
            base = t * TT
            for k in range(K):
                nc.tensor.matmul(
                    pc[:, 0:TT],
                    lhsT=wtbf_sb[0:64, k * C : (k + 1) * C],
                    rhs=x_b[0:64, base + k : base + k + TT],
                    start=(k == 0), stop=(k == K - 1),
                    skip_group_check=True,
                )
                nc.tensor.matmul(
                    pc[:, TT : 2 * TT],
                    lhsT=wtbf_sb[64:128, k * C : (k + 1) * C],
                    rhs=x_b[64:128, base + k : base + k + TT],
                    start=(k == 0), stop=(k == K - 1),
                    skip_group_check=True,
                )
            st = statp.tile([C, 2 * TT], FP32, tag="st")
            nc.scalar.copy(st[:], pc[:])
            return st

        def emit_conv(b, x_b, wsc, start_tile=0, hook_a=None, hook_pa=None,
                      hook_b=None, hook_pb=None, post_pair=None):
            """5 shifted matmuls per 512-tile; 2 PSUM banks per ACT copy.
            hook_a fires after pair hook_pa's matmuls; hook_b fires between
            the halves of pair hook_pb; post_pair[i] fires after pair i's
            PSUM->SBUF copy."""
            last = b == BL - 1
            pair_idx = -1
            for g in range(NTILES // GRP):
                t0g = g * GRP
                if t0g + GRP <= start_tile:
                    continue
                o_sb = outp.tile([C, GRP * TT], BF16, tag="osb")
                for jj in range(max(t0g, start_tile), t0g + GRP, 2):
                    pair_idx += 1
                    pc = pconv.tile([C, 2 * TT], FP32, tag="pc")
                    for half in range(2):
                        if hook_b is not None and pair_idx == hook_pb and half == 1:
                            hook_b()
                        j = jj + half
                        for k in range(K):
                            nc.tensor.matmul(
                                pc[:, half * TT : (half + 1) * TT],
                                lhsT=wsc[:, k * C : (k + 1) * C],
                                rhs=x_b[:, j * TT + k : j * TT + k + TT],
                                start=(k == 0),
                                stop=(k == K - 1),
                            )
                    if hook_a is not None and pair_idx == hook_pa:
                        hook_a()
                    off = (jj - t0g) * TT
                    if last and g == NTILES // GRP - 1:
                        # drain the final sample's output in 1-tile pieces so
                        # the kernel tail isn't gated on one large copy+DMA
                        for q in range(2):
                            nc.scalar.copy(
                                o_sb[:, off + q * TT : off + (q + 1) * TT],
                                pc[:, q * TT : (q + 1) * TT],
                            )
                            nc.scalar.dma_start(
                                out_d[b, :, t0g * TT + off + q * TT : t0g * TT + off + (q + 1) * TT],
                                o_sb[:, off + q * TT : off + (q + 1) * TT],
                            )
                    else:
                        nc.scalar.copy(o_sb[:, off : off + 2 * TT], pc[:])
                    if post_pair is not None and pair_idx in post_pair:
                        post_pair[pair_idx]()
                if not (last and g == NTILES // GRP - 1):
                    nc.scalar.dma_start(
                        out_d[b, :, t0g * TT : (t0g + GRP) * TT], o_sb[:]
                    )

        # ================= sample 0: fill with the static path =================
        x0 = xb.tile([C, T + 2 * PAD], BF16, tag="xb")
        nc.vector.memset(x0[:, 0:PAD], 0.0)
        nc.vector.memset(x0[:, T + PAD : T + 2 * PAD], 0.0)
        parts0 = small.tile([C, len(CH0) - 1], FP32, tag="gapp")
        for c in range(len(CH0) - 1):
            nc.sync.dma_start(
                x0[:, PAD + CH0[c] : PAD + CH0[c + 1]], x_d[0, :, CH0[c] : CH0[c + 1]]
            )

        # sums + static tiles, interleaved so ACT's sums precede its copies
        sum_dve(parts0, 0, x0, CH0[0], CH0[1])
        sum_act(parts0, 1, x0, CH0[1], CH0[2])
        sts = [emit_static_tile(0, x0)]
        sum_dve(parts0, 2, x0, CH0[2], CH0[3])
        sum_act(parts0, 3, x0, CH0[3], CH0[4])
        sts.append(emit_static_tile(1, x0))
        sum_dve(parts0, 4, x0, CH0[4], CH0[5])
        sum_dve(parts0, 5, x0, CH0[5], CH0[6])
        sts.append(emit_static_tile(2, x0))
        sts.append(emit_static_tile(3, x0))
        gap0 = emit_gap_finalize(parts0)
        sts.append(emit_static_tile(4, x0))
        hps0 = emit_h_matmul(gap0)
        hpre0 = emit_h_pre(0, hps0)
        sts.append(emit_static_tile(5, x0))
        hsb0 = emit_sigmoid(hpre0)
        sts.append(emit_static_tile(6, x0))
        wsc0, sps0 = emit_S_wsc(hsb0, want_hcols=True)
        # h columns + ratio h1/h0 to SBUF for the combines
        hcols_sb = small.tile([C, 2], FP32, tag="h0sb")
        nc.vector.tensor_copy(hcols_sb[:], sps0[:, 128:130])
        h0sb = hcols_sb[:, 0:1]
        r_sb = small.tile([C, 1], FP32, tag="rsb")
        nc.vector.reciprocal(r_sb[:], hcols_sb[:, 0:1])
        nc.vector.tensor_mul(r_sb[:], r_sb[:], hcols_sb[:, 1:2])
        sts.append(emit_static_tile(7, x0))

        # sample 1 load + row-sum shares
        x1 = xb.tile([C, T + 2 * PAD], BF16, tag="xb")
        nc.vector.memset(x1[:, 0:PAD], 0.0)
        nc.vector.memset(x1[:, T + PAD : T + 2 * PAD], 0.0)
        nc.sync.dma_start(x1[:, PAD : PAD + T], x_d[1, :, 0:T])
        parts1 = small.tile([C, 2], FP32, tag="gapp")
        sum_dve(parts1, 0, x1, 0, SPLIT)
        sum_act(parts1, 1, x1, SPLIT, T)

        state = {"parts": parts1}

        def mk_hook_a(bn):
            def h():
                gap = emit_gap_finalize(state["parts"])
                hps = emit_h_matmul(gap)
                state["hpre"] = emit_h_pre(bn, hps)
            return h

        def mk_hook_b(bn):
            def h():
                hsb = emit_sigmoid(state["hpre"])
                wsc_n, _ = emit_S_wsc(hsb, want_hcols=False)
                state["wsc_next"] = wsc_n
            return h

        # conv(0): normal path for tiles 8-15, sample-1 h chain hooked in
        emit_conv(
            0, x0, wsc0, start_tile=S_TILES,
            hook_a=mk_hook_a(1), hook_pa=2,
            hook_b=mk_hook_b(1), hook_pb=3,
        )
        wsc1 = state["wsc_next"]

        # static-combine DVE halves: u_t = st0 + r*st1
        us = []
        for t in range(S_TILES):
            u = small.tile([C, TT], FP32, tag="ctmp", bufs=8)
            nc.vector.scalar_tensor_tensor(
                u[:], sts[t][:, TT : 2 * TT], r_sb[:, 0:1], sts[t][:, 0:TT],
                op0=ALU.mult, op1=ALU.add,
            )
            us.append(u)
        osb0 = outp.tile([C, GRP * TT], BF16, tag="osb")
        osb1 = outp.tile([C, GRP * TT], BF16, tag="osb")

        def combacts(lo, hi, o_sb, dma_lo):
            def f():
                for t in range(lo, hi):
                    nc.scalar.activation(
                        o_sb[:, (t - lo) * TT : (t - lo + 1) * TT], us[t][:],
                        AF.Copy, scale=h0sb[:, 0:1],
                    )
                nc.scalar.dma_start(
                    out_d[0, :, dma_lo : dma_lo + GRP * TT], o_sb[:]
                )
            return f

        # ---- samples 1..3 ----
        x_cur, wsc_cur = x1, wsc1
        for b in range(1, BL):
            post = {}
            if b == 1:
                post[0] = combacts(0, 4, osb0, 0)
                post[1] = combacts(4, 8, osb1, GRP * TT)
            if b + 1 < BL:
                x_n = xb.tile([C, T + 2 * PAD], BF16, tag="xb")
                nc.vector.memset(x_n[:, 0:PAD], 0.0)
                nc.vector.memset(x_n[:, T + PAD : T + 2 * PAD], 0.0)
                nc.sync.dma_start(x_n[:, PAD : PAD + T], x_d[b + 1, :, 0:T])
                parts_n = small.tile([C, 2], FP32, tag="gapp")
                sum_dve(parts_n, 0, x_n, 0, SPLIT)
                state["parts"] = parts_n
                state["xn"] = x_n

                def mk_sum_act(pn, xn):
                    return lambda: sum_act(pn, 1, xn, SPLIT, T)
                post[2] = mk_sum_act(parts_n, x_n)
                emit_conv(
                    b, x_cur, wsc_cur,
                    hook_a=mk_hook_a(b + 1), hook_pa=5,
                    hook_b=mk_hook_b(b + 1), hook_pb=6,
                    post_pair=post,
                )
                x_cur, wsc_cur = state["xn"], state["wsc_next"]
            else:
                emit_conv(b, x_cur, wsc_cur, post_pair=post)

    nc.compile()
    return nc


_NC_CACHE = None


def _get_nc():
    global _NC_CACHE
    if _NC_CACHE is None:
        _NC_CACHE = build_nc()
    return _NC_CACHE


def make_in_maps(x, input_lengths, w1, b1, w2):
    import ml_dtypes

    xbf = np.asarray(x, dtype=np.float32).astype(ml_dtypes.bfloat16)
    lens = np.asarray(input_lengths).astype(np.float64)
    invlen = (1.0 / lens).astype(np.float32)
    w1t = np.ascontiguousarray(
        np.asarray(w1, dtype=np.float32).T.astype(ml_dtypes.bfloat16)
    )  # (C, 2C) bf16
    b1r = np.asarray(b1, dtype=np.float32).reshape(1, 2 * C)
    # wt[ci, k*C + co] = W[co, ci, k],  W = w2.reshape(C, C, K)
    wt = np.ascontiguousarray(
        np.asarray(w2, dtype=np.float32)
        .reshape(C, C, K)
        .transpose(1, 2, 0)
        .reshape(C, K * C)
    )
    wtbf = wt.astype(ml_dtypes.bfloat16)
    ones = np.ones((1, 64), dtype=ml_dtypes.bfloat16)

    in_maps = []
    for i in range(NCORES):
        sl = slice(i * BL, (i + 1) * BL)
        in_maps.append(
            {
                "x": np.ascontiguousarray(xbf[sl]),
                "invlen": np.ascontiguousarray(invlen[sl].reshape(1, BL)),
                "w1t": w1t,
                "b1": b1r,
                "wt": wt,
                "wtbf": wtbf,
                "ones": ones,
            }
        )
    return in_maps


def kernel(x, input_lengths, w1, b1, w2, _trace=False):
    nc = _get_nc()
    in_maps = make_in_maps(x, input_lengths, w1, b1, w2)
    res = run_bass_kernel_spmd(nc, in_maps, core_ids=list(range(NCORES)), trace=_trace)
    out = np.concatenate(
        [res.results[i]["out"].astype(np.float32) for i in range(NCORES)], axis=0
    )
    if _trace:
        kernel.last_exec_time_ns = res.exec_time_ns
        kernel.last_results = res
    return out


# revision 14
# speedup vs baseline: 1.0493x; 1.0088x over previous
"""ContextNet dynamic-conv kernel for 8 TRN2 NeuronCores.

Math: the reference computes, per sample b:
    gap[b]  = x[b].sum(T) / len[b]                  (C,)
    h[b]    = sigmoid(gap[b] @ w1.T + b1)           (2C,)
    w_dyn[b, co, ci, k] = h[b, 2*co + (ci>=C/2)] * W[co, ci, k]
        where W = w2.reshape(C, C, K)               (static across batch!)
    out[b]  = conv1d(x[b], w_dyn[b], pad=K//2)      (C, T)

Key structure: the per-sample weight is a static tensor W scaled by
h-factors that depend only on (output channel, input-channel half):
    out[b] = h0[co] * conv(x[b,:64], W[:, :64]) + h1[co] * conv(x[b,64:], W[:, 64:])

Two conv paths exploit this:
  * normal path: scale the pre-transposed W by S_b once per sample (bf16)
    and run the conv as 5 shifted full-128-contraction bf16 matmuls per
    512-col tile, accumulating in PSUM.
  * static path (pipeline fill only): sample 0's first S_TILES tiles are
    computed with the UNSCALED weight halves as two interleaved 64-deep
    row-tiled matmul groups (tile_position (0,0)/(64,0), concurrent in
    the PE sub-arrays), staged to SBUF in f32, and combined with h per
    output channel AFTER h is ready:
        out = (st0 + (h1/h0)*st1) * h0
    one DVE scalar_tensor_tensor + one ACT scaled-copy per tile.  The PE
    starts as soon as the first x chunk lands instead of waiting for the
    full sample + h chain, and warms the HAM clock gate.

GAP row sums cost ~1 ns/elem/lane on every engine (no engine is faster),
so each sample's 8192-elem sum is split between DVE (tensor_reduce) and
ACT (activation Copy + accum_out).  All h-chain and helper ops are
hand-placed into the per-engine FIFOs (instruction queues are strict
program order) so no queue ever stalls another through backpressure.

x ships from the host as bf16 (the conv is bf16 anyway): halves input
HBM traffic and the sample-0 critical load.  Output leaves as bf16.

Sharding: pure data parallel over batch B=32 -> 4 samples per core x 8.
"""

import numpy as np
from contextlib import ExitStack

import concourse.bacc as bacc
import concourse.tile as tile
from concourse import mybir
from concourse.bass_utils import run_bass_kernel_spmd

B, C, T = 32, 128, 8192
K = 5
PAD = (K - 1) // 2
NCORES = 8
BL = B // NCORES          # samples per core
TT = 512                  # conv tile width (one PSUM bank of f32)
NTILES = T // TT
GRP = 4                   # conv tiles batched per output DMA (512 KiB bf16)
S_TILES = 8               # sample-0 tiles on the static (split-conv) path
# sample-0 input chunk bounds (halo-aligned to 512*j + 4 so static tiles
# unlock as chunks land; later samples load in one 2 MiB DMA)
CH0 = [0, 1028, 3076, 5636, 7172, 7684, 8192]
CH0_ACT = (1, 3)          # chunks row-summed on ACT (rest on DVE)
# b>=1 row-sum slices: [0:SPLA) DVE unguarded, [SPLA:SPLB) DVE ordered
# after the previous sample's weight scaling, [SPLB:T) ACT
SPLA = 3072
SPLB = 6144
N_WARM = 5                # dummy matmuls to start warming the PE clock gate

FP32 = mybir.dt.float32
BF16 = mybir.dt.bfloat16

AF = mybir.ActivationFunctionType
ALU = mybir.AluOpType
AXL = mybir.AxisListType


def build_nc():
    nc = bacc.Bacc("TRN2", target_bir_lowering=False, debug=False)

    x_d = nc.dram_tensor("x", [BL, C, T], BF16, kind="ExternalInput").ap()
    il_d = nc.dram_tensor("invlen", [1, BL], FP32, kind="ExternalInput").ap()
    w1t_d = nc.dram_tensor("w1t", [C, 2 * C], BF16, kind="ExternalInput").ap()
    b1_d = nc.dram_tensor("b1", [1, 2 * C], FP32, kind="ExternalInput").ap()
    wt_d = nc.dram_tensor("wt", [C, K * C], FP32, kind="ExternalInput").ap()
    wtbf_d = nc.dram_tensor("wtbf", [C, K * C], BF16, kind="ExternalInput").ap()
    ones_d = nc.dram_tensor("ones", [1, 64], BF16, kind="ExternalInput").ap()
    out_d = nc.dram_tensor("out", [BL, C, T], BF16, kind="ExternalOutput").ap()

    with ExitStack() as ctx:
        tc = ctx.enter_context(tile.TileContext(nc))

        const = ctx.enter_context(tc.tile_pool(name="const", bufs=1))
        xb = ctx.enter_context(tc.tile_pool(name="xb", bufs=3))
        statp = ctx.enter_context(tc.tile_pool(name="statp", bufs=S_TILES))
        outp = ctx.enter_context(tc.tile_pool(name="outp", bufs=6))
        small = ctx.enter_context(tc.tile_pool(name="small", bufs=3))
        wscp = ctx.enter_context(tc.tile_pool(name="wscp", bufs=2))
        pconv = ctx.enter_context(tc.tile_pool(name="pconv", bufs=3, space="PSUM"))
        psp = ctx.enter_context(tc.tile_pool(name="psp", bufs=1, space="PSUM"))
        php = ctx.enter_context(tc.tile_pool(name="php", bufs=1, space="PSUM"))

        # constants ride the ACT HWDGE ring; wtbf first (static MMs need it
        # earliest), wt (only needed for the first weight scaling) last
        dz = const.tile([C, TT], BF16)
        nc.vector.memset(dz[:], 0.0)
        wtbf_sb = const.tile([C, K * C], BF16)
        nc.scalar.dma_start(wtbf_sb[:], wtbf_d[:])
        w1t_sb = const.tile([C, 2 * C], BF16)
        nc.scalar.dma_start(w1t_sb[:], w1t_d[:])
        b1_sb = const.tile([1, 2 * C], FP32)
        nc.scalar.dma_start(b1_sb[:], b1_d[:])
        il_sb = const.tile([1, BL], FP32)
        nc.scalar.dma_start(il_sb[:], il_d[:])
        ones_sb = const.tile([1, 64], BF16)
        nc.scalar.dma_start(ones_sb[:], ones_d[:])
        wt_sb = const.tile([C, K * C], FP32)
        nc.scalar.dma_start(wt_sb[:], wt_d[:])
        trash = const.tile([C, 2048], BF16)   # ACT accum_out row-sum byproduct

        # dummy sigmoid so ACT loads its function table during the fill,
        # not in the middle of sample 0's h chain
        sgs = const.tile([1, 1], FP32)
        nc.scalar.activation(sgs[:], dz[0:1, 0:1], AF.Sigmoid)

        # HAM warm-up: matmuls on zeroed SBUF into the psp scratch bank.
        pwarm = psp.tile([C, TT], FP32, tag="s")
        for _ in range(N_WARM):
            nc.tensor.matmul(pwarm[:], lhsT=dz[:, 0:C], rhs=dz[:], start=True, stop=True)

        # ---------------- schedulable pieces ----------------
        def sum_dve(parts, col, x_b, lo, hi):
            nc.vector.tensor_reduce(
                parts[:, col : col + 1], x_b[:, PAD + lo : PAD + hi],
                axis=AXL.X, op=ALU.add,
            )

        def sum_act(parts, col, x_b, lo, hi):
            nc.scalar.activation(
                trash[:, 0 : hi - lo], x_b[:, PAD + lo : PAD + hi], AF.Copy,
                accum_out=parts[:, col : col + 1],
            )

        def emit_gap_finalize(parts):
            gap_r = small.tile([C, 1], FP32, tag="gapr")
            nc.vector.tensor_reduce(gap_r[:], parts[:], axis=AXL.X, op=ALU.add)
            gap_bf = small.tile([C, 1], BF16, tag="gapbf")
            nc.vector.tensor_copy(gap_bf[:], gap_r[:])
            return gap_bf

        def emit_h_matmul(gap_bf):
            h_ps = php.tile([1, 2 * C], FP32, tag="h")
            nc.tensor.matmul(
                h_ps[:], lhsT=gap_bf[:], rhs=w1t_sb[:], start=True, stop=True
            )
            return h_ps

        def emit_h_pre(b, h_ps):
            h_pre = small.tile([1, 2 * C], FP32, tag="hpre")
            nc.vector.scalar_tensor_tensor(
                h_pre[:], h_ps[:], il_sb[0:1, b : b + 1], b1_sb[:],
                op0=ALU.mult, op1=ALU.add,
            )
            return h_pre

        def emit_sigmoid(h_pre):
            h_sb = small.tile([1, 2 * C], BF16, tag="h")
            nc.scalar.activation(h_sb[:], h_pre[:], AF.Sigmoid)
            return h_sb

        def emit_S_wsc(h_sb, want_hcols=False):
            """S broadcast matmuls + weight scaling (h columns first so the
            PE is done with the bank before DVE reads it)."""
            h3 = h_sb[:].rearrange("p (a two) -> p two a", two=2)  # (1, 2, 128)
            s_ps = psp.tile([C, 130], FP32, tag="s")
            if want_hcols:
                nc.tensor.matmul(
                    s_ps[:, 128:129], lhsT=h3[:, 0, :], rhs=ones_sb[0:1, 0:1],
                    start=True, stop=True,
                )
                nc.tensor.matmul(
                    s_ps[:, 129:130], lhsT=h3[:, 1, :], rhs=ones_sb[0:1, 0:1],
                    start=True, stop=True,
                )
            # S_b[ci, co] = h[2co + (ci>=64)] via contract-1 broadcast
            nc.tensor.matmul(
                s_ps[0:64, 0:128], lhsT=ones_sb[:], rhs=h3[:, 0, :],
                start=True, stop=True,
            )
            nc.tensor.matmul(
                s_ps[64:128, 0:128], lhsT=ones_sb[:], rhs=h3[:, 1, :],
                start=True, stop=True,
            )
            wsc = wscp.tile([C, K * C], BF16, tag="wsc")
            for k in range(K):
                nc.vector.tensor_mul(
                    wsc[:, k * C : (k + 1) * C],
                    wt_sb[:, k * C : (k + 1) * C],
                    s_ps[:, 0:128],
                )
            return wsc, s_ps

        def emit_static_tile(t, x_b):
            """One 512-col tile of sample 0 via two interleaved 64-deep
            row-tiled matmul groups (concurrent PE sub-arrays)."""
            pc = pconv.tile([C, 2 * TT], FP32, tag="pc")
            base = t * TT
            for k in range(K):
                nc.tensor.matmul(
                    pc[:, 0:TT],
                    lhsT=wtbf_sb[0:64, k * C : (k + 1) * C],
                    rhs=x_b[0:64, base + k : base + k + TT],
                    start=(k == 0), stop=(k == K - 1),
                    skip_group_check=True,
                )
                nc.tensor.matmul(
                    pc[:, TT : 2 * TT],
                    lhsT=wtbf_sb[64:128, k * C : (k + 1) * C],
                    rhs=x_b[64:128, base + k : base + k + TT],
                    start=(k == 0), stop=(k == K - 1),
                    skip_group_check=True,
                )
            st = statp.tile([C, 2 * TT], FP32, tag="st")
            nc.scalar.copy(st[:], pc[:])
            return st

        def emit_conv(b, x_b, wsc, start_tile=0, hook_a=None, hook_pa=None,
                      hook_b=None, hook_pb=None, post_pair=None):
            """5 shifted matmuls per 512-tile; 2 PSUM banks per ACT copy.
            hook_a fires after pair hook_pa's matmuls; hook_b fires between
            the halves of pair hook_pb; post_pair[i] fires after pair i's
            PSUM->SBUF copy."""
            last = b == BL - 1
            pair_idx = -1
            for g in range(NTILES // GRP):
                t0g = g * GRP
                if t0g + GRP <= start_tile:
                    continue
                o_sb = outp.tile([C, GRP * TT], BF16, tag="osb")
                for jj in range(max(t0g, start_tile), t0g + GRP, 2):
                    pair_idx += 1
                    pc = pconv.tile([C, 2 * TT], FP32, tag="pc")
                    for half in range(2):
                        if hook_b is not None and pair_idx == hook_pb and half == 1:
                            hook_b()
                        j = jj + half
                        for k in range(K):
                            nc.tensor.matmul(
                                pc[:, half * TT : (half + 1) * TT],
                                lhsT=wsc[:, k * C : (k + 1) * C],
                                rhs=x_b[:, j * TT + k : j * TT + k + TT],
                                start=(k == 0),
                                stop=(k == K - 1),
                            )
                    if hook_a is not None and pair_idx == hook_pa:
                        hook_a()
                    off = (jj - t0g) * TT
                    if last and g == NTILES // GRP - 1:
                        # drain the final sample's output in 1-tile pieces so
                        # the kernel tail isn't gated on one large copy+DMA
                        for q in range(2):
                            nc.scalar.copy(
                                o_sb[:, off + q * TT : off + (q + 1) * TT],
                                pc[:, q * TT : (q + 1) * TT],
                            )
                            nc.scalar.dma_start(
                                out_d[b, :, t0g * TT + off + q * TT : t0g * TT + off + (q + 1) * TT],
                                o_sb[:, off + q * TT : off + (q + 1) * TT],
                            )
                    else:
                        nc.scalar.copy(o_sb[:, off : off + 2 * TT], pc[:])
                    if post_pair is not None and pair_idx in post_pair:
                        post_pair[pair_idx]()
                if not (last and g == NTILES // GRP - 1):
                    nc.scalar.dma_start(
                        out_d[b, :, t0g * TT : (t0g + GRP) * TT], o_sb[:]
                    )

        # ================= sample 0: fill with the static path =================
        x0 = xb.tile([C, T + 2 * PAD], BF16, tag="xb")
        nc.vector.memset(x0[:, 0:PAD], 0.0)
        nc.vector.memset(x0[:, T + PAD : T + 2 * PAD], 0.0)
        parts0 = small.tile([C, len(CH0) - 1], FP32, tag="gapp")
        for c in range(len(CH0) - 1):
            nc.sync.dma_start(
                x0[:, PAD + CH0[c] : PAD + CH0[c + 1]], x_d[0, :, CH0[c] : CH0[c + 1]]
            )

        # sums + static tiles, interleaved so ACT's sums precede its copies
        sum_dve(parts0, 0, x0, CH0[0], CH0[1])
        sum_act(parts0, 1, x0, CH0[1], CH0[2])
        sts = [emit_static_tile(0, x0)]
        sum_dve(parts0, 2, x0, CH0[2], CH0[3])
        sum_act(parts0, 3, x0, CH0[3], CH0[4])
        sts.append(emit_static_tile(1, x0))
        sum_dve(parts0, 4, x0, CH0[4], CH0[5])
        sum_dve(parts0, 5, x0, CH0[5], CH0[6])
        sts.append(emit_static_tile(2, x0))
        sts.append(emit_static_tile(3, x0))
        gap0 = emit_gap_finalize(parts0)
        sts.append(emit_static_tile(4, x0))
        hps0 = emit_h_matmul(gap0)
        hpre0 = emit_h_pre(0, hps0)
        sts.append(emit_static_tile(5, x0))
        hsb0 = emit_sigmoid(hpre0)
        sts.append(emit_static_tile(6, x0))
        wsc0, sps0 = emit_S_wsc(hsb0, want_hcols=True)
        # h columns + ratio h1/h0 to SBUF for the combines
        hcols_sb = small.tile([C, 2], FP32, tag="h0sb")
        nc.vector.tensor_copy(hcols_sb[:], sps0[:, 128:130])
        h0sb = hcols_sb[:, 0:1]
        r_sb = small.tile([C, 1], FP32, tag="rsb")
        nc.vector.reciprocal(r_sb[:], hcols_sb[:, 0:1])
        nc.vector.tensor_mul(r_sb[:], r_sb[:], hcols_sb[:, 1:2])
        sts.append(emit_static_tile(7, x0))

        def emit_loadn(b, wsc_prev):
            """Samples 1..: one 2 MiB DMA; row sum in three slices.  The
            middle DVE slice reads a token from the previous sample's wsc
            so the scheduler cannot hoist the bulk reduce ahead of the tiny
            critical weight-scaling ops on the DVE queue."""
            x_b = xb.tile([C, T + 2 * PAD], BF16, tag="xb")
            nc.vector.memset(x_b[:, 0:PAD], 0.0)
            nc.vector.memset(x_b[:, T + PAD : T + 2 * PAD], 0.0)
            nc.sync.dma_start(x_b[:, PAD : PAD + T], x_d[b, :, 0:T])
            parts = small.tile([C, 3], FP32, tag="gapp")
            sum_dve(parts, 0, x_b, 0, SPLA)
            wsc_tok = wsc_prev[0:1, :].rearrange("p (k c) -> p k c", k=K)[:, :, 0]
            nc.vector.tensor_reduce(parts[0:1, 1:2], wsc_tok, axis=AXL.X, op=ALU.add)
            sum_dve(parts, 1, x_b, SPLA, SPLB)
            sum_act(parts, 2, x_b, SPLB, T)
            return x_b, parts

        # sample 1 load + row-sum shares
        x1, parts1 = emit_loadn(1, wsc0)

        state = {"parts": parts1}

        def mk_hook_a(bn):
            def h():
                gap = emit_gap_finalize(state["parts"])
                hps = emit_h_matmul(gap)
                state["hpre"] = emit_h_pre(bn, hps)
            return h

        def mk_hook_b(bn):
            def h():
                hsb = emit_sigmoid(state["hpre"])
                wsc_n, _ = emit_S_wsc(hsb, want_hcols=False)
                state["wsc_next"] = wsc_n
            return h

        # conv(0): normal path for tiles 8-15, sample-1 h chain hooked in
        emit_conv(
            0, x0, wsc0, start_tile=S_TILES,
            hook_a=mk_hook_a(1), hook_pa=2,
            hook_b=mk_hook_b(1), hook_pb=3,
        )
        wsc1 = state["wsc_next"]

        # static-combine DVE halves: u_t = st0 + r*st1
        us = []
        for t in range(S_TILES):
            u = small.tile([C, TT], FP32, tag="ctmp", bufs=8)
            nc.vector.scalar_tensor_tensor(
                u[:], sts[t][:, TT : 2 * TT], r_sb[:, 0:1], sts[t][:, 0:TT],
                op0=ALU.mult, op1=ALU.add,
            )
            us.append(u)
        osb0 = outp.tile([C, GRP * TT], BF16, tag="osb")
        osb1 = outp.tile([C, GRP * TT], BF16, tag="osb")

        def combacts(lo, hi, o_sb, dma_lo):
            def f():
                for t in range(lo, hi):
                    nc.scalar.activation(
                        o_sb[:, (t - lo) * TT : (t - lo + 1) * TT], us[t][:],
                        AF.Copy, scale=h0sb[:, 0:1],
                    )
                nc.scalar.dma_start(
                    out_d[0, :, dma_lo : dma_lo + GRP * TT], o_sb[:]
                )
            return f

        # ---- samples 1..3 ----
        x_cur, wsc_cur = x1, wsc1
        for b in range(1, BL):
            post = {}
            if b == 1:
                post[0] = combacts(0, 4, osb0, 0)
                post[1] = combacts(4, 8, osb1, GRP * TT)
            if b + 1 < BL:
                x_n, parts_n = emit_loadn(b + 1, wsc_cur)
                state["parts"] = parts_n
                state["xn"] = x_n
                emit_conv(
                    b, x_cur, wsc_cur,
                    hook_a=mk_hook_a(b + 1), hook_pa=5,
                    hook_b=mk_hook_b(b + 1), hook_pb=6,
                    post_pair=post,
                )
                x_cur, wsc_cur = state["xn"], state["wsc_next"]
            else:
                emit_conv(b, x_cur, wsc_cur, post_pair=post)

    nc.compile()
    return nc


_NC_CACHE = None


def _get_nc():
    global _NC_CACHE
    if _NC_CACHE is None:
        _NC_CACHE = build_nc()
    return _NC_CACHE


def make_in_maps(x, input_lengths, w1, b1, w2):
    import ml_dtypes

    xbf = np.asarray(x, dtype=np.float32).astype(ml_dtypes.bfloat16)
    lens = np.asarray(input_lengths).astype(np.float64)
    invlen = (1.0 / lens).astype(np.float32)
    w1t = np.ascontiguousarray(
        np.asarray(w1, dtype=np.float32).T.astype(ml_dtypes.bfloat16)
    )  # (C, 2C) bf16
    b1r = np.asarray(b1, dtype=np.float32).reshape(1, 2 * C)
    # wt[ci, k*C + co] = W[co, ci, k],  W = w2.reshape(C, C, K)
    wt = np.ascontiguousarray(
        np.asarray(w2, dtype=np.float32)
        .reshape(C, C, K)
        .transpose(1, 2, 0)
        .reshape(C, K * C)
    )
    wtbf = wt.astype(ml_dtypes.bfloat16)
    ones = np.ones((1, 64), dtype=ml_dtypes.bfloat16)

    in_maps = []
    for i in range(NCORES):
        sl = slice(i * BL, (i + 1) * BL)
        in_maps.append(
            {
                "x": np.ascontiguousarray(xbf[sl]),
                "invlen": np.ascontiguousarray(invlen[sl].reshape(1, BL)),
                "w1t": w1t,
                "b1": b1r,
                "wt": wt,
                "wtbf": wtbf,
                "ones": ones,
            }
        )
    return in_maps


def kernel(x, input_lengths, w1, b1, w2, _trace=False):
    nc = _get_nc()
    in_maps = make_in_maps(x, input_lengths, w1, b1, w2)
    res = run_bass_kernel_spmd(nc, in_maps, core_ids=list(range(NCORES)), trace=_trace)
    out = np.concatenate(
        [res.results[i]["out"].astype(np.float32) for i in range(NCORES)], axis=0
    )
    if _trace:
        kernel.last_exec_time_ns = res.exec_time_ns
        kernel.last_results = res
    return out


# revision 20
# speedup vs baseline: 1.0552x; 1.0056x over previous
"""ContextNet dynamic-conv kernel for 8 TRN2 NeuronCores.

Math: the reference computes, per sample b:
    gap[b]  = x[b].sum(T) / len[b]                  (C,)
    h[b]    = sigmoid(gap[b] @ w1.T + b1)           (2C,)
    w_dyn[b, co, ci, k] = h[b, 2*co + (ci>=C/2)] * W[co, ci, k]
        where W = w2.reshape(C, C, K)               (static across batch!)
    out[b]  = conv1d(x[b], w_dyn[b], pad=K//2)      (C, T)

Key structure: the per-sample weight is a static tensor W scaled by
h-factors that depend only on (output channel, input-channel half):
    out[b] = h0[co] * conv(x[b,:64], W[:, :64]) + h1[co] * conv(x[b,64:], W[:, 64:])

Two conv paths exploit this:
  * normal path: scale the pre-transposed W by S_b once per sample (bf16)
    and run the conv as 5 shifted full-128-contraction bf16 matmuls per
    512-col tile, accumulating in PSUM.
  * static path (pipeline fill only): sample 0's first S_TILES tiles are
    computed with the UNSCALED weight halves as two interleaved 64-deep
    row-tiled matmul groups (tile_position (0,0)/(64,0), concurrent in
    the PE sub-arrays), staged to SBUF in f32, and combined with h per
    output channel AFTER h is ready:
        out = (st0 + (h1/h0)*st1) * h0
    one DVE scalar_tensor_tensor + one ACT scaled-copy per tile.  The PE
    starts as soon as the first x chunk lands instead of waiting for the
    full sample + h chain, and warms the HAM clock gate.

GAP row sums cost ~1 ns/elem/lane on every engine (no engine is faster),
so each sample's 8192-elem sum is split between DVE (tensor_reduce) and
ACT (activation Copy + accum_out).  All h-chain and helper ops are
hand-placed into the per-engine FIFOs (instruction queues are strict
program order) so no queue ever stalls another through backpressure.

x ships from the host as bf16 (the conv is bf16 anyway): halves input
HBM traffic and the sample-0 critical load.  Output leaves as bf16.

Sharding: pure data parallel over batch B=32 -> 4 samples per core x 8.
"""

import numpy as np
from contextlib import ExitStack

import concourse.bacc as bacc
import concourse.tile as tile
from concourse import mybir
from concourse.bass_utils import run_bass_kernel_spmd

B, C, T = 32, 128, 8192
K = 5
PAD = (K - 1) // 2
NCORES = 8
BL = B // NCORES          # samples per core
TT = 512                  # conv tile width (one PSUM bank of f32)
NTILES = T // TT
GRP = 4                   # conv tiles batched per output DMA (512 KiB bf16)
S_TILES = 8               # sample-0 tiles on the static (split-conv) path
# sample-0 input chunk bounds (halo-aligned to 512*j + 4 so static tiles
# unlock as chunks land; later samples load in one 2 MiB DMA)
CH0 = [0, 1028, 3076, 5636, 7172, 7684, 8192]
# b>=1 row-sum split: [0:SPLB) DVE (ordered after the previous sample's
# weight scaling via a token read), [SPLB:T) ACT
SPLB = 6144
N_WARM = 5                # dummy matmuls to start warming the PE clock gate

FP32 = mybir.dt.float32
BF16 = mybir.dt.bfloat16

AF = mybir.ActivationFunctionType
ALU = mybir.AluOpType
AXL = mybir.AxisListType


def build_nc():
    nc = bacc.Bacc("TRN2", target_bir_lowering=False, debug=False)

    x_d = nc.dram_tensor("x", [BL, C, T], BF16, kind="ExternalInput").ap()
    il_d = nc.dram_tensor("invlen", [1, BL], FP32, kind="ExternalInput").ap()
    w1t_d = nc.dram_tensor("w1t", [C, 2 * C], BF16, kind="ExternalInput").ap()
    b1_d = nc.dram_tensor("b1", [1, 2 * C], FP32, kind="ExternalInput").ap()
    wt_d = nc.dram_tensor("wt", [C, K * C], FP32, kind="ExternalInput").ap()
    wtbf_d = nc.dram_tensor("wtbf", [C, K * C], BF16, kind="ExternalInput").ap()
    ones_d = nc.dram_tensor("ones", [1, 64], BF16, kind="ExternalInput").ap()
    out_d = nc.dram_tensor("out", [BL, C, T], BF16, kind="ExternalOutput").ap()

    with ExitStack() as ctx:
        tc = ctx.enter_context(tile.TileContext(nc))

        const = ctx.enter_context(tc.tile_pool(name="const", bufs=1))
        xb = ctx.enter_context(tc.tile_pool(name="xb", bufs=3))
        statp = ctx.enter_context(tc.tile_pool(name="statp", bufs=S_TILES))
        outp = ctx.enter_context(tc.tile_pool(name="outp", bufs=6))
        small = ctx.enter_context(tc.tile_pool(name="small", bufs=3))
        wscp = ctx.enter_context(tc.tile_pool(name="wscp", bufs=2))
        pconv = ctx.enter_context(tc.tile_pool(name="pconv", bufs=3, space="PSUM"))
        psp = ctx.enter_context(tc.tile_pool(name="psp", bufs=1, space="PSUM"))
        php = ctx.enter_context(tc.tile_pool(name="php", bufs=1, space="PSUM"))

        # constants ride the ACT HWDGE ring; wtbf first (static MMs need it
        # earliest), wt (only needed for the first weight scaling) last
        dz = const.tile([C, TT], BF16)
        nc.vector.memset(dz[:], 0.0)
        wtbf_sb = const.tile([C, K * C], BF16)
        nc.scalar.dma_start(wtbf_sb[:], wtbf_d[:])
        w1t_sb = const.tile([C, 2 * C], BF16)
        nc.scalar.dma_start(w1t_sb[:], w1t_d[:])
        b1_sb = const.tile([1, 2 * C], FP32)
        nc.scalar.dma_start(b1_sb[:], b1_d[:])
        il_sb = const.tile([1, BL], FP32)
        nc.scalar.dma_start(il_sb[:], il_d[:])
        ones_sb = const.tile([1, 64], BF16)
        nc.scalar.dma_start(ones_sb[:], ones_d[:])
        wt_sb = const.tile([C, K * C], FP32)
        nc.scalar.dma_start(wt_sb[:], wt_d[:])
        trash = const.tile([C, 2048], BF16)   # ACT accum_out row-sum byproduct

        # dummy sigmoid so ACT loads its function table during the fill,
        # not in the middle of sample 0's h chain
        sgs = const.tile([1, 1], FP32)
        nc.scalar.activation(sgs[:], dz[0:1, 0:1], AF.Sigmoid)

        # HAM warm-up: matmuls on zeroed SBUF into the psp scratch bank.
        pwarm = psp.tile([C, TT], FP32, tag="s")
        for _ in range(N_WARM):
            nc.tensor.matmul(pwarm[:], lhsT=dz[:, 0:C], rhs=dz[:], start=True, stop=True)

        # ---------------- schedulable pieces ----------------
        def sum_dve(parts, col, x_b, lo, hi):
            nc.vector.tensor_reduce(
                parts[:, col : col + 1], x_b[:, PAD + lo : PAD + hi],
                axis=AXL.X, op=ALU.add,
            )

        def sum_act(parts, col, x_b, lo, hi):
            nc.scalar.activation(
                trash[:, 0 : hi - lo], x_b[:, PAD + lo : PAD + hi], AF.Copy,
                accum_out=parts[:, col : col + 1],
            )

        def emit_gap_finalize(parts):
            gap_r = small.tile([C, 1], FP32, tag="gapr")
            nc.vector.tensor_reduce(gap_r[:], parts[:], axis=AXL.X, op=ALU.add)
            gap_bf = small.tile([C, 1], BF16, tag="gapbf")
            nc.vector.tensor_copy(gap_bf[:], gap_r[:])
            return gap_bf

        def emit_h_matmul(gap_bf):
            h_ps = php.tile([1, 2 * C], FP32, tag="h")
            nc.tensor.matmul(
                h_ps[:], lhsT=gap_bf[:], rhs=w1t_sb[:], start=True, stop=True
            )
            return h_ps

        def emit_h_pre(b, h_ps):
            h_pre = small.tile([1, 2 * C], FP32, tag="hpre")
            nc.vector.scalar_tensor_tensor(
                h_pre[:], h_ps[:], il_sb[0:1, b : b + 1], b1_sb[:],
                op0=ALU.mult, op1=ALU.add,
            )
            return h_pre

        def emit_sigmoid(h_pre):
            h_sb = small.tile([1, 2 * C], BF16, tag="h")
            nc.scalar.activation(h_sb[:], h_pre[:], AF.Sigmoid)
            return h_sb

        def emit_S_wsc(h_sb, want_hcols=False):
            """S broadcast matmuls + weight scaling (h columns first so the
            PE is done with the bank before DVE reads it)."""
            h3 = h_sb[:].rearrange("p (a two) -> p two a", two=2)  # (1, 2, 128)
            s_ps = psp.tile([C, 130], FP32, tag="s")
            if want_hcols:
                nc.tensor.matmul(
                    s_ps[:, 128:129], lhsT=h3[:, 0, :], rhs=ones_sb[0:1, 0:1],
                    start=True, stop=True,
                )
                nc.tensor.matmul(
                    s_ps[:, 129:130], lhsT=h3[:, 1, :], rhs=ones_sb[0:1, 0:1],
                    start=True, stop=True,
                )
            # S_b[ci, co] = h[2co + (ci>=64)] via contract-1 broadcast
            nc.tensor.matmul(
                s_ps[0:64, 0:128], lhsT=ones_sb[:], rhs=h3[:, 0, :],
                start=True, stop=True,
            )
            nc.tensor.matmul(
                s_ps[64:128, 0:128], lhsT=ones_sb[:], rhs=h3[:, 1, :],
                start=True, stop=True,
            )
            wsc = wscp.tile([C, K * C], BF16, tag="wsc")
            for k in range(K):
                nc.vector.tensor_mul(
                    wsc[:, k * C : (k + 1) * C],
                    wt_sb[:, k * C : (k + 1) * C],
                    s_ps[:, 0:128],
                )
            return wsc, s_ps

        def emit_static_tile(t, x_b):
            """One 512-col tile of sample 0 via two interleaved 64-deep
            row-tiled matmul groups (concurrent PE sub-arrays)."""
            pc = pconv.tile([C, 2 * TT], FP32, tag="pc")
            base = t * TT
            for k in range(K):
                nc.tensor.matmul(
                    pc[:, 0:TT],
                    lhsT=wtbf_sb[0:64, k * C : (k + 1) * C],
                    rhs=x_b[0:64, base + k : base + k + TT],
                    start=(k == 0), stop=(k == K - 1),
                    skip_group_check=True,
                )
                nc.tensor.matmul(
                    pc[:, TT : 2 * TT],
                    lhsT=wtbf_sb[64:128, k * C : (k + 1) * C],
                    rhs=x_b[64:128, base + k : base + k + TT],
                    start=(k == 0), stop=(k == K - 1),
                    skip_group_check=True,
                )
            st = statp.tile([C, 2 * TT], FP32, tag="st")
            nc.scalar.copy(st[:], pc[:])
            return st

        def emit_conv(b, x_b, wsc, start_tile=0, hook_a=None, hook_pa=None,
                      hook_b=None, hook_pb=None, post_pair=None):
            """5 shifted matmuls per 512-tile; 2 PSUM banks per ACT copy.
            hook_a fires between the halves of pair hook_pa; hook_b fires
            after pair hook_pb's matmuls; post_pair[i] fires after pair i's
            PSUM->SBUF copy."""
            last = b == BL - 1
            pair_idx = -1
            for g in range(NTILES // GRP):
                t0g = g * GRP
                if t0g + GRP <= start_tile:
                    continue
                o_sb = outp.tile([C, GRP * TT], BF16, tag="osb")
                for jj in range(max(t0g, start_tile), t0g + GRP, 2):
                    pair_idx += 1
                    pc = pconv.tile([C, 2 * TT], FP32, tag="pc")
                    for half in range(2):
                        if hook_a is not None and pair_idx == hook_pa and half == 1:
                            hook_a()
                        j = jj + half
                        for k in range(K):
                            nc.tensor.matmul(
                                pc[:, half * TT : (half + 1) * TT],
                                lhsT=wsc[:, k * C : (k + 1) * C],
                                rhs=x_b[:, j * TT + k : j * TT + k + TT],
                                start=(k == 0),
                                stop=(k == K - 1),
                            )
                    if hook_b is not None and pair_idx == hook_pb:
                        hook_b()
                    off = (jj - t0g) * TT
                    if last and g == NTILES // GRP - 1:
                        # drain the final sample's output in 1-tile pieces so
                        # the kernel tail isn't gated on one large copy+DMA
                        for q in range(2):
                            nc.scalar.copy(
                                o_sb[:, off + q * TT : off + (q + 1) * TT],
                                pc[:, q * TT : (q + 1) * TT],
                            )
                            nc.scalar.dma_start(
                                out_d[b, :, t0g * TT + off + q * TT : t0g * TT + off + (q + 1) * TT],
                                o_sb[:, off + q * TT : off + (q + 1) * TT],
                            )
                    else:
                        nc.scalar.copy(o_sb[:, off : off + 2 * TT], pc[:])
                    if post_pair is not None and pair_idx in post_pair:
                        post_pair[pair_idx]()
                if not (last and g == NTILES // GRP - 1):
                    nc.scalar.dma_start(
                        out_d[b, :, t0g * TT : (t0g + GRP) * TT], o_sb[:]
                    )

        # ================= sample 0: fill with the static path =================
        x0 = xb.tile([C, T + 2 * PAD], BF16, tag="xb")
        nc.vector.memset(x0[:, 0:PAD], 0.0)
        nc.vector.memset(x0[:, T + PAD : T + 2 * PAD], 0.0)
        parts0 = small.tile([C, len(CH0) - 1], FP32, tag="gapp")
        for c in range(len(CH0) - 1):
            nc.sync.dma_start(
                x0[:, PAD + CH0[c] : PAD + CH0[c + 1]], x_d[0, :, CH0[c] : CH0[c + 1]]
            )

        # all sample-0 row sums on DVE: ACT does only the static-tile PSUM
        # copies during the fill so they pace the PE without backpressure
        sts = []
        for c in range(len(CH0) - 1):
            sum_dve(parts0, c, x0, CH0[c], CH0[c + 1])
            if c < 2:
                sts.append(emit_static_tile(c, x0))
        sts.append(emit_static_tile(2, x0))
        sts.append(emit_static_tile(3, x0))
        gap0 = emit_gap_finalize(parts0)
        sts.append(emit_static_tile(4, x0))
        sts.append(emit_static_tile(5, x0))
        hps0 = emit_h_matmul(gap0)
        hpre0 = emit_h_pre(0, hps0)
        hsb0 = emit_sigmoid(hpre0)
        sts.append(emit_static_tile(6, x0))
        wsc0, sps0 = emit_S_wsc(hsb0, want_hcols=True)
        # h columns + ratio h1/h0 to SBUF for the combines
        hcols_sb = small.tile([C, 2], FP32, tag="h0sb")
        nc.vector.tensor_copy(hcols_sb[:], sps0[:, 128:130])
        h0sb = hcols_sb[:, 0:1]
        r_sb = small.tile([C, 1], FP32, tag="rsb")
        nc.vector.reciprocal(r_sb[:], hcols_sb[:, 0:1])
        nc.vector.tensor_mul(r_sb[:], r_sb[:], hcols_sb[:, 1:2])
        sts.append(emit_static_tile(7, x0))

        def emit_loadn(b, wsc_prev):
            """Samples 1..: one 2 MiB DMA; row sum in two slices.  The DVE
            slice first reads a token from the previous sample's wsc so the
            scheduler cannot hoist the bulk reduce ahead of the tiny
            critical weight-scaling ops on the DVE queue."""
            x_b = xb.tile([C, T + 2 * PAD], BF16, tag="xb")
            nc.vector.memset(x_b[:, 0:PAD], 0.0)
            nc.vector.memset(x_b[:, T + PAD : T + 2 * PAD], 0.0)
            nc.sync.dma_start(x_b[:, PAD : PAD + T], x_d[b, :, 0:T])
            parts = small.tile([C, 2], FP32, tag="gapp")
            wsc_tok = wsc_prev[0:1, :].rearrange("p (k c) -> p k c", k=K)[:, :, 0]
            nc.vector.tensor_reduce(parts[0:1, 0:1], wsc_tok, axis=AXL.X, op=ALU.add)
            sum_dve(parts, 0, x_b, 0, SPLB)
            sum_act(parts, 1, x_b, SPLB, T)
            return x_b, parts

        # sample 1 load + row-sum shares
        x1, parts1 = emit_loadn(1, wsc0)

        state = {"parts": parts1}

        def mk_hook_a(bn):
            def h():
                gap = emit_gap_finalize(state["parts"])
                hps = emit_h_matmul(gap)
                state["hpre"] = emit_h_pre(bn, hps)
            return h

        def mk_hook_b(bn):
            def h():
                hsb = emit_sigmoid(state["hpre"])
                wsc_n, _ = emit_S_wsc(hsb, want_hcols=False)
                state["wsc_next"] = wsc_n
            return h

        # conv(0): normal path for tiles 8-15, sample-1 h chain hooked in
        emit_conv(
            0, x0, wsc0, start_tile=S_TILES,
            hook_a=mk_hook_a(1), hook_pa=3,
            hook_b=mk_hook_b(1), hook_pb=3,
        )
        wsc1 = state["wsc_next"]

        # static-combine DVE halves: u_t = st0 + r*st1
        us = []
        for t in range(S_TILES):
            u = small.tile([C, TT], FP32, tag="ctmp", bufs=8)
            nc.vector.scalar_tensor_tensor(
                u[:], sts[t][:, TT : 2 * TT], r_sb[:, 0:1], sts[t][:, 0:TT],
                op0=ALU.mult, op1=ALU.add,
            )
            us.append(u)
        osb0 = outp.tile([C, GRP * TT], BF16, tag="osb")
        osb1 = outp.tile([C, GRP * TT], BF16, tag="osb")

        def combacts(lo, hi, o_sb, dma_lo):
            def f():
                for t in range(lo, hi):
                    nc.scalar.activation(
                        o_sb[:, (t - lo) * TT : (t - lo + 1) * TT], us[t][:],
                        AF.Copy, scale=h0sb[:, 0:1],
                    )
                nc.scalar.dma_start(
                    out_d[0, :, dma_lo : dma_lo + GRP * TT], o_sb[:]
                )
            return f

        # ---- samples 1..3 ----
        x_cur, wsc_cur = x1, wsc1
        for b in range(1, BL):
            post = {}
            if b == 1:
                post[0] = combacts(0, 4, osb0, 0)
                post[1] = combacts(4, 8, osb1, GRP * TT)
            if b + 1 < BL:
                x_n, parts_n = emit_loadn(b + 1, wsc_cur)
                state["parts"] = parts_n
                state["xn"] = x_n
                emit_conv(
                    b, x_cur, wsc_cur,
                    hook_a=mk_hook_a(b + 1), hook_pa=6,
                    hook_b=mk_hook_b(b + 1), hook_pb=6,
                    post_pair=post,
                )
                x_cur, wsc_cur = state["xn"], state["wsc_next"]
            else:
                emit_conv(b, x_cur, wsc_cur, post_pair=post)

    nc.compile()
    return nc


_NC_CACHE = None


def _get_nc():
    global _NC_CACHE
    if _NC_CACHE is None:
        _NC_CACHE = build_nc()
    return _NC_CACHE


def make_in_maps(x, input_lengths, w1, b1, w2):
    import ml_dtypes

    xbf = np.asarray(x, dtype=np.float32).astype(ml_dtypes.bfloat16)
    lens = np.asarray(input_lengths).astype(np.float64)
    invlen = (1.0 / lens).astype(np.float32)
    w1t = np.ascontiguousarray(
        np.asarray(w1, dtype=np.float32).T.astype(ml_dtypes.bfloat16)
    )  # (C, 2C) bf16
    b1r = np.asarray(b1, dtype=np.float32).reshape(1, 2 * C)
    # wt[ci, k*C + co] = W[co, ci, k],  W = w2.reshape(C, C, K)
    wt = np.ascontiguousarray(
        np.asarray(w2, dtype=np.float32)
        .reshape(C, C, K)
        .transpose(1, 2, 0)
        .reshape(C, K * C)
    )
    wtbf = wt.astype(ml_dtypes.bfloat16)
    ones = np.ones((1, 64), dtype=ml_dtypes.bfloat16)

    in_maps = []
    for i in range(NCORES):
        sl = slice(i * BL, (i + 1) * BL)
        in_maps.append(
            {
                "x": np.ascontiguousarray(xbf[sl]),
                "invlen": np.ascontiguousarray(invlen[sl].reshape(1, BL)),
                "w1t": w1t,
                "b1": b1r,
                "wt": wt,
                "wtbf": wtbf,
                "ones": ones,
            }
        )
    return in_maps


def kernel(x, input_lengths, w1, b1, w2, _trace=False):
    nc = _get_nc()
    in_maps = make_in_maps(x, input_lengths, w1, b1, w2)
    res = run_bass_kernel_spmd(nc, in_maps, core_ids=list(range(NCORES)), trace=_trace)
    out = np.concatenate(
        [res.results[i]["out"].astype(np.float32) for i in range(NCORES)], axis=0
    )
    if _trace:
        kernel.last_exec_time_ns = res.exec_time_ns
        kernel.last_results = res
    return out


# revision 30
# speedup vs baseline: 1.0720x; 1.0159x over previous
"""ContextNet dynamic-conv kernel for 8 TRN2 NeuronCores.

Math: the reference computes, per sample b:
    gap[b]  = x[b].sum(T) / len[b]                  (C,)
    h[b]    = sigmoid(gap[b] @ w1.T + b1)           (2C,)
    w_dyn[b, co, ci, k] = h[b, 2*co + (ci>=C/2)] * W[co, ci, k]
        where W = w2.reshape(C, C, K)               (static across batch!)
    out[b]  = conv1d(x[b], w_dyn[b], pad=K//2)      (C, T)

Key structure: the per-sample weight is a static tensor W scaled by
h-factors that depend only on (output channel, input-channel half):
    out[b] = h0[co] * conv(x[b,:64], W[:, :64]) + h1[co] * conv(x[b,64:], W[:, 64:])

Two conv paths exploit this:
  * normal path: scale the pre-transposed W by S_b once per sample (bf16)
    and run the conv as 5 shifted full-128-contraction bf16 matmuls per
    512-col tile, accumulating in PSUM.
  * static path (pipeline fill only): sample 0's first S_TILES tiles are
    computed with the UNSCALED weight halves as two interleaved 64-deep
    row-tiled matmul groups (tile_position (0,0)/(64,0), concurrent in
    the PE sub-arrays), staged to SBUF in f32, and combined with h per
    output channel AFTER h is ready:
        out = (st0 + (h1/h0)*st1) * h0
    one DVE scalar_tensor_tensor + one ACT scaled-copy per tile.  The PE
    starts as soon as the first x chunk lands instead of waiting for the
    full sample + h chain, and warms the HAM clock gate.

GAP row sums cost ~1 ns/elem/lane on every engine (no engine is faster),
so each sample's 8192-elem sum is split between DVE (tensor_reduce) and
ACT (activation Copy + accum_out).  All h-chain and helper ops are
hand-placed into the per-engine FIFOs (instruction queues are strict
program order) so no queue ever stalls another through backpressure.

x ships from the host as bf16 (the conv is bf16 anyway): halves input
HBM traffic and the sample-0 critical load.  Output leaves as bf16.

Sharding: pure data parallel over batch B=32 -> 4 samples per core x 8.
"""

import numpy as np
from contextlib import ExitStack

import concourse.bacc as bacc
import concourse.tile as tile
from concourse import mybir
from concourse.bass_utils import run_bass_kernel_spmd

B, C, T = 32, 128, 8192
K = 5
PAD = (K - 1) // 2
NCORES = 8
BL = B // NCORES          # samples per core
TT = 512                  # conv tile width (one PSUM bank of f32)
NTILES = T // TT
GRP = 4                   # conv tiles batched per output DMA (512 KiB bf16)
S_TILES = 8               # sample-0 tiles on the static (split-conv) path
# sample-0 input chunk bounds (halo-aligned to 512*j + 4 so static tiles
# unlock as chunks land; later samples load in one 2 MiB DMA)
CH0 = [0, 1028, 3076, 5636, 7172, 7684, 8192]
# b>=1 row-sum slices: [0:SPLA) DVE unguarded, [SPLA:SPLB) DVE ordered
# after the previous sample's weight scaling, [SPLB:T) ACT ordered after
# the previous sample's sigmoid (the scheduler's DMA model is optimistic,
# so unordered bulk sums get hoisted ahead of the critical chain ops)
SPLA = 3072
SPLB = 6144
N_WARM = 5                # dummy matmuls to start warming the PE clock gate

FP32 = mybir.dt.float32
BF16 = mybir.dt.bfloat16

AF = mybir.ActivationFunctionType
ALU = mybir.AluOpType
AXL = mybir.AxisListType


def build_nc():
    nc = bacc.Bacc("TRN2", target_bir_lowering=False, debug=False)

    x_d = nc.dram_tensor("x", [BL, C, T], BF16, kind="ExternalInput").ap()
    il_d = nc.dram_tensor("invlen", [1, BL], FP32, kind="ExternalInput").ap()
    w1t_d = nc.dram_tensor("w1t", [C, 2 * C], BF16, kind="ExternalInput").ap()
    b1_d = nc.dram_tensor("b1", [1, 2 * C], FP32, kind="ExternalInput").ap()
    wt_d = nc.dram_tensor("wt", [C, K * C], FP32, kind="ExternalInput").ap()
    wtbf_d = nc.dram_tensor("wtbf", [C, K * C], BF16, kind="ExternalInput").ap()
    ones_d = nc.dram_tensor("ones", [1, 64], BF16, kind="ExternalInput").ap()
    out_d = nc.dram_tensor("out", [BL, C, T], BF16, kind="ExternalOutput").ap()

    with ExitStack() as ctx:
        tc = ctx.enter_context(tile.TileContext(nc))

        const = ctx.enter_context(tc.tile_pool(name="const", bufs=1))
        xb = ctx.enter_context(tc.tile_pool(name="xb", bufs=3))
        statp = ctx.enter_context(tc.tile_pool(name="statp", bufs=S_TILES))
        outp = ctx.enter_context(tc.tile_pool(name="outp", bufs=6))
        small = ctx.enter_context(tc.tile_pool(name="small", bufs=3))
        wscp = ctx.enter_context(tc.tile_pool(name="wscp", bufs=2))
        pconv = ctx.enter_context(tc.tile_pool(name="pconv", bufs=3, space="PSUM"))
        psp = ctx.enter_context(tc.tile_pool(name="psp", bufs=1, space="PSUM"))
        php = ctx.enter_context(tc.tile_pool(name="php", bufs=1, space="PSUM"))

        # constants ride the ACT HWDGE ring; wtbf first (static MMs need it
        # earliest), wt (only needed for the first weight scaling) last
        dz = const.tile([C, TT], BF16)
        nc.vector.memset(dz[:], 0.0)
        wtbf_sb = const.tile([C, K * C], BF16)
        nc.scalar.dma_start(wtbf_sb[:], wtbf_d[:])
        w1t_sb = const.tile([C, 2 * C], BF16)
        b1_sb = const.tile([1, 2 * C], FP32)
        il_sb = const.tile([1, BL], FP32)
        ones_sb = const.tile([1, 64], BF16)
        wt_sb = const.tile([C, K * C], FP32)
        trash = const.tile([C, 2048], BF16)   # ACT accum_out row-sum byproduct

        # dummy sigmoid so ACT loads its function table during the fill,
        # not in the middle of sample 0's h chain
        sgs = const.tile([1, 1], FP32)
        nc.scalar.activation(sgs[:], dz[0:1, 0:1], AF.Sigmoid)

        # HAM warm-up: matmuls on zeroed SBUF into the psp scratch bank.
        pwarm = psp.tile([C, TT], FP32, tag="s")
        for _ in range(N_WARM):
            nc.tensor.matmul(pwarm[:], lhsT=dz[:, 0:C], rhs=dz[:], start=True, stop=True)

        # ---------------- schedulable pieces ----------------
        def sum_dve(parts, col, x_b, lo, hi):
            nc.vector.tensor_reduce(
                parts[:, col : col + 1], x_b[:, PAD + lo : PAD + hi],
                axis=AXL.X, op=ALU.add,
            )

        def sum_act(parts, col, x_b, lo, hi):
            nc.scalar.activation(
                trash[:, 0 : hi - lo], x_b[:, PAD + lo : PAD + hi], AF.Copy,
                accum_out=parts[:, col : col + 1],
            )

        def emit_gap_finalize(parts):
            gap_r = small.tile([C, 1], FP32, tag="gapr")
            nc.vector.tensor_reduce(gap_r[:], parts[:], axis=AXL.X, op=ALU.add)
            gap_bf = small.tile([C, 1], BF16, tag="gapbf")
            nc.vector.tensor_copy(gap_bf[:], gap_r[:])
            return gap_bf

        def emit_h_matmul(gap_bf):
            h_ps = php.tile([1, 2 * C], FP32, tag="h")
            nc.tensor.matmul(
                h_ps[:], lhsT=gap_bf[:], rhs=w1t_sb[:], start=True, stop=True
            )
            return h_ps

        def emit_h_pre(b, h_ps):
            h_pre = small.tile([1, 2 * C], FP32, tag="hpre")
            nc.vector.scalar_tensor_tensor(
                h_pre[:], h_ps[:], il_sb[0:1, b : b + 1], b1_sb[:],
                op0=ALU.mult, op1=ALU.add,
            )
            return h_pre

        def emit_sigmoid(h_pre):
            h_sb = small.tile([1, 2 * C], BF16, tag="h")
            nc.scalar.activation(h_sb[:], h_pre[:], AF.Sigmoid)
            return h_sb

        def emit_S_wsc(h_sb, want_hcols=False):
            """S broadcast matmuls + weight scaling (h columns first so the
            PE is done with the bank before DVE reads it)."""
            h3 = h_sb[:].rearrange("p (a two) -> p two a", two=2)  # (1, 2, 128)
            s_ps = psp.tile([C, 130], FP32, tag="s")
            if want_hcols:
                nc.tensor.matmul(
                    s_ps[:, 128:129], lhsT=h3[:, 0, :], rhs=ones_sb[0:1, 0:1],
                    start=True, stop=True,
                )
                nc.tensor.matmul(
                    s_ps[:, 129:130], lhsT=h3[:, 1, :], rhs=ones_sb[0:1, 0:1],
                    start=True, stop=True,
                )
            # S_b[ci, co] = h[2co + (ci>=64)] via contract-1 broadcast
            nc.tensor.matmul(
                s_ps[0:64, 0:128], lhsT=ones_sb[:], rhs=h3[:, 0, :],
                start=True, stop=True,
            )
            nc.tensor.matmul(
                s_ps[64:128, 0:128], lhsT=ones_sb[:], rhs=h3[:, 1, :],
                start=True, stop=True,
            )
            wsc = wscp.tile([C, K * C], BF16, tag="wsc")
            for k in range(K):
                nc.vector.tensor_mul(
                    wsc[:, k * C : (k + 1) * C],
                    wt_sb[:, k * C : (k + 1) * C],
                    s_ps[:, 0:128],
                )
            return wsc, s_ps

        def emit_static_tile(t, x_b):
            """One 512-col tile of sample 0 via two interleaved 64-deep
            row-tiled matmul groups (concurrent PE sub-arrays)."""
            pc = pconv.tile([C, 2 * TT], FP32, tag="pc")
            base = t * TT
            for k in range(K):
                nc.tensor.matmul(
                    pc[:, 0:TT],
                    lhsT=wtbf_sb[0:64, k * C : (k + 1) * C],
                    rhs=x_b[0:64, base + k : base + k + TT],
                    start=(k == 0), stop=(k == K - 1),
                    skip_group_check=True,
                )
                nc.tensor.matmul(
                    pc[:, TT : 2 * TT],
                    lhsT=wtbf_sb[64:128, k * C : (k + 1) * C],
                    rhs=x_b[64:128, base + k : base + k + TT],
                    start=(k == 0), stop=(k == K - 1),
                    skip_group_check=True,
                )
            st = statp.tile([C, 2 * TT], FP32, tag="st")
            nc.scalar.copy(st[:], pc[:])
            return st

        def emit_conv(b, x_b, wsc, start_tile=0, hook_a=None, hook_pa=None,
                      hook_b=None, hook_pb=None, post_pair=None):
            """5 shifted matmuls per 512-tile; 2 PSUM banks per ACT copy.
            hook_a fires between the halves of pair hook_pa; hook_b fires
            after pair hook_pb's matmuls; post_pair[i] fires after pair i's
            PSUM->SBUF copy."""
            last = b == BL - 1
            pair_idx = -1
            for g in range(NTILES // GRP):
                t0g = g * GRP
                if t0g + GRP <= start_tile:
                    continue
                o_sb = outp.tile([C, GRP * TT], BF16, tag="osb")
                for jj in range(max(t0g, start_tile), t0g + GRP, 2):
                    pair_idx += 1
                    pc = pconv.tile([C, 2 * TT], FP32, tag="pc")
                    for half in range(2):
                        if hook_a is not None and pair_idx == hook_pa and half == 1:
                            hook_a()
                        j = jj + half
                        for k in range(K):
                            nc.tensor.matmul(
                                pc[:, half * TT : (half + 1) * TT],
                                lhsT=wsc[:, k * C : (k + 1) * C],
                                rhs=x_b[:, j * TT + k : j * TT + k + TT],
                                start=(k == 0),
                                stop=(k == K - 1),
                            )
                    if hook_b is not None and pair_idx == hook_pb:
                        hook_b()
                    off = (jj - t0g) * TT
                    if last and g == NTILES // GRP - 1:
                        # drain the final sample's output in 1-tile pieces,
                        # alternating copy engine (ACT/DVE) and DMA ring so
                        # the kernel tail pipelines instead of serializing
                        for q in range(2):
                            dst = o_sb[:, off + q * TT : off + (q + 1) * TT]
                            src = pc[:, q * TT : (q + 1) * TT]
                            lo = t0g * TT + off + q * TT
                            if q == 0:
                                nc.scalar.copy(dst, src)
                                nc.scalar.dma_start(out_d[b, :, lo : lo + TT], dst)
                            else:
                                nc.vector.tensor_copy(dst, src)
                                nc.sync.dma_start(out_d[b, :, lo : lo + TT], dst)
                    else:
                        nc.scalar.copy(o_sb[:, off : off + 2 * TT], pc[:])
                    if post_pair is not None and pair_idx in post_pair:
                        post_pair[pair_idx]()
                if not (last and g == NTILES // GRP - 1):
                    nc.scalar.dma_start(
                        out_d[b, :, t0g * TT : (t0g + GRP) * TT], o_sb[:]
                    )

        # ================= sample 0: fill with the static path =================
        x0 = xb.tile([C, T + 2 * PAD], BF16, tag="xb")
        nc.vector.memset(x0[:, 0:PAD], 0.0)
        nc.vector.memset(x0[:, T + PAD : T + 2 * PAD], 0.0)
        parts0 = small.tile([C, len(CH0) - 1], FP32, tag="gapp")
        # chunks alternate between the two HWDGE rings so descriptor issue
        # (~0.6 us per dma_start, serial per sequencer) pipelines
        for c in range(len(CH0) - 1):
            eng = nc.sync if c % 2 == 0 else nc.scalar
            eng.dma_start(
                x0[:, PAD + CH0[c] : PAD + CH0[c + 1]], x_d[0, :, CH0[c] : CH0[c + 1]]
            )
        # remaining small constants follow the odd chunks on the ACT ring;
        # wt (f32, only needed for weight scaling at ~20 us) queues on the
        # sync ring AFTER sample 0's chunks so it doesn't steal early HBM
        # bandwidth from the fill-critical load
        nc.scalar.dma_start(w1t_sb[:], w1t_d[:])
        nc.scalar.dma_start(b1_sb[:], b1_d[:])
        nc.scalar.dma_start(il_sb[:], il_d[:])
        nc.scalar.dma_start(ones_sb[:], ones_d[:])
        nc.sync.dma_start(wt_sb[:], wt_d[:])

        # all sample-0 row sums on DVE: ACT does only the static-tile PSUM
        # copies during the fill so they pace the PE without backpressure
        sts = []
        for c in range(len(CH0) - 1):
            sum_dve(parts0, c, x0, CH0[c], CH0[c + 1])
            if c < 2:
                sts.append(emit_static_tile(c, x0))
        sts.append(emit_static_tile(2, x0))
        sts.append(emit_static_tile(3, x0))
        gap0 = emit_gap_finalize(parts0)
        sts.append(emit_static_tile(4, x0))
        sts.append(emit_static_tile(5, x0))
        hps0 = emit_h_matmul(gap0)
        hpre0 = emit_h_pre(0, hps0)
        hsb0 = emit_sigmoid(hpre0)
        sts.append(emit_static_tile(6, x0))
        wsc0, sps0 = emit_S_wsc(hsb0, want_hcols=True)
        # h columns + ratio h1/h0 to SBUF for the combines
        hcols_sb = small.tile([C, 2], FP32, tag="h0sb")
        nc.vector.tensor_copy(hcols_sb[:], sps0[:, 128:130])
        h0sb = hcols_sb[:, 0:1]
        r_sb = small.tile([C, 1], FP32, tag="rsb")
        nc.vector.reciprocal(r_sb[:], hcols_sb[:, 0:1])
        nc.vector.tensor_mul(r_sb[:], r_sb[:], hcols_sb[:, 1:2])
        sts.append(emit_static_tile(7, x0))

        def emit_loadn(b, wsc_prev, hsb_prev):
            """Samples 1..: one 2 MiB DMA; row sum in three slices.  Token
            reads order the bulk sums after the previous sample's critical
            chain ops on each engine queue."""
            x_b = xb.tile([C, T + 2 * PAD], BF16, tag="xb")
            nc.vector.memset(x_b[:, 0:PAD], 0.0)
            nc.vector.memset(x_b[:, T + PAD : T + 2 * PAD], 0.0)
            nc.sync.dma_start(x_b[:, PAD : PAD + T], x_d[b, :, 0:T])
            parts = small.tile([C, 3], FP32, tag="gapp")
            sum_dve(parts, 0, x_b, 0, SPLA)
            wsc_tok = wsc_prev[0:1, :].rearrange("p (k c) -> p k c", k=K)[:, :, 0]
            nc.vector.tensor_reduce(parts[0:1, 1:2], wsc_tok, axis=AXL.X, op=ALU.add)
            sum_dve(parts, 1, x_b, SPLA, SPLB)
            nc.scalar.copy(trash[0:1, 0:1], hsb_prev[0:1, 0:1])
            sum_act(parts, 2, x_b, SPLB, T)
            return x_b, parts

        # sample 1 load + row-sum shares
        x1, parts1 = emit_loadn(1, wsc0, hsb0)

        state = {"parts": parts1}

        def mk_hook_a(bn):
            def h():
                gap = emit_gap_finalize(state["parts"])
                hps = emit_h_matmul(gap)
                state["hpre"] = emit_h_pre(bn, hps)
            return h

        def mk_hook_b(bn):
            def h():
                hsb = emit_sigmoid(state["hpre"])
                wsc_n, _ = emit_S_wsc(hsb, want_hcols=False)
                state["wsc_next"] = wsc_n
                state["hsb"] = hsb
            return h

        # conv(0): normal path for tiles 8-15, sample-1 h chain hooked in
        emit_conv(
            0, x0, wsc0, start_tile=S_TILES,
            hook_a=mk_hook_a(1), hook_pa=3,
            hook_b=mk_hook_b(1), hook_pb=3,
        )
        wsc1 = state["wsc_next"]

        # static-combine DVE halves: u_t = st0 + r*st1
        us = []
        for t in range(S_TILES):
            u = small.tile([C, TT], FP32, tag="ctmp", bufs=8)
            nc.vector.scalar_tensor_tensor(
                u[:], sts[t][:, TT : 2 * TT], r_sb[:, 0:1], sts[t][:, 0:TT],
                op0=ALU.mult, op1=ALU.add,
            )
            us.append(u)
        osb0 = outp.tile([C, GRP * TT], BF16, tag="osb")
        osb1 = outp.tile([C, GRP * TT], BF16, tag="osb")

        def combacts(lo, hi, o_sb, dma_lo):
            def f():
                for t in range(lo, hi):
                    nc.scalar.activation(
                        o_sb[:, (t - lo) * TT : (t - lo + 1) * TT], us[t][:],
                        AF.Copy, scale=h0sb[:, 0:1],
                    )
                nc.scalar.dma_start(
                    out_d[0, :, dma_lo : dma_lo + GRP * TT], o_sb[:]
                )
            return f

        # ---- samples 1..3 ----
        x_cur, wsc_cur = x1, wsc1
        for b in range(1, BL):
            post = {}
            if b == 1:
                post[0] = combacts(0, 4, osb0, 0)
                post[1] = combacts(4, 8, osb1, GRP * TT)
            if b + 1 < BL:
                x_n, parts_n = emit_loadn(b + 1, wsc_cur, state["hsb"])
                state["parts"] = parts_n
                state["xn"] = x_n
                emit_conv(
                    b, x_cur, wsc_cur,
                    hook_a=mk_hook_a(b + 1), hook_pa=6,
                    hook_b=mk_hook_b(b + 1), hook_pb=6,
                    post_pair=post,
                )
                x_cur, wsc_cur = state["xn"], state["wsc_next"]
            else:
                emit_conv(b, x_cur, wsc_cur, post_pair=post)

    nc.compile()
    return nc


_NC_CACHE = None


def _get_nc():
    global _NC_CACHE
    if _NC_CACHE is None:
        _NC_CACHE = build_nc()
    return _NC_CACHE


def make_in_maps(x, input_lengths, w1, b1, w2):
    import ml_dtypes

    xbf = np.asarray(x, dtype=np.float32).astype(ml_dtypes.bfloat16)
    lens = np.asarray(input_lengths).astype(np.float64)
    invlen = (1.0 / lens).astype(np.float32)
    w1t = np.ascontiguousarray(
        np.asarray(w1, dtype=np.float32).T.astype(ml_dtypes.bfloat16)
    )  # (C, 2C) bf16
    b1r = np.asarray(b1, dtype=np.float32).reshape(1, 2 * C)
    # wt[ci, k*C + co] = W[co, ci, k],  W = w2.reshape(C, C, K)
    wt = np.ascontiguousarray(
        np.asarray(w2, dtype=np.float32)
        .reshape(C, C, K)
        .transpose(1, 2, 0)
        .reshape(C, K * C)
    )
    wtbf = wt.astype(ml_dtypes.bfloat16)
    ones = np.ones((1, 64), dtype=ml_dtypes.bfloat16)

    in_maps = []
    for i in range(NCORES):
        sl = slice(i * BL, (i + 1) * BL)
        in_maps.append(
            {
                "x": np.ascontiguousarray(xbf[sl]),
                "invlen": np.ascontiguousarray(invlen[sl].reshape(1, BL)),
                "w1t": w1t,
                "b1": b1r,
                "wt": wt,
                "wtbf": wtbf,
                "ones": ones,
            }
        )
    return in_maps


def kernel(x, input_lengths, w1, b1, w2, _trace=False):
    nc = _get_nc()
    in_maps = make_in_maps(x, input_lengths, w1, b1, w2)
    res = run_bass_kernel_spmd(nc, in_maps, core_ids=list(range(NCORES)), trace=_trace)
    out = np.concatenate(
        [res.results[i]["out"].astype(np.float32) for i in range(NCORES)], axis=0
    )
    if _trace:
        kernel.last_exec_time_ns = res.exec_time_ns
        kernel.last_results = res
    return out
